# revision 43
# baseline (speedup 1.0000x reference)
"""Trainium2 Bass kernel for nn_DAGLinkPredictor (3-layer GAT + edge decoder).

Sharding: dst-node-sharded GAT across 8 cores. Edges (incl self-loops) are
sorted by dst and grouped into per-core 128-node dst blocks. Per block:
  - dma_gather pulls [h | als] rows (bf16) for edge sources from the node
    table T_l (two gathers: src < 32768 and >= 32768, int16 gather indices),
  - dst-side [als|ald] comes from compact local DRAM tables (256B rows)
    via dma_gather (an S^T@ald PE-matmul alternative measured 2x slower on
    real HW: per-chunk DVE->PE->Act->PE semaphore chains),
  - a one-hot scatter matrix S (VectorE is_equal) scatter-adds messages into
    PSUM via TensorE,
  - softmax is denominator-style: out[d] = sum_e exp(lrelu(als+ald)) * h[src]
    / sum_e exp(...) (exactly segment-softmax; logits are O(1) so no max).
T1 is computed fully on every core from the replicated input (no collective,
prologue inputs pre-permuted to the chunk-major table layout). T2/T3/TD
slices are AllGathered in 2 contiguous chunks overlapped with the block
loop. Gathers round-robin over 4 SWDGE queues (the serialized per-call
completion wait was the dominant real-HW cost). Node-phase matmuls are
bf16.
Decoder: transpose-gathers of z rows + dense matmuls, trans_bias via
gathered rows x one-hot reduce.
"""
import numpy as np
import ml_dtypes

N = 50000
NP = 50176            # padded nodes: 8 * 6272
SLICE = NP // 8       # 6272 nodes per core
NB = SLICE // 128     # 49 blocks per core
NBF = NP // 128       # 392 blocks in the full table
E = 800000
EL = 100000
HALF = 32768          # int16 gather index limit
NTYPES = 311
EMB = 16
COMB = 48

# layer configs: (Din, HD, H, D, src_elem(row stride), dst_off_elems)
LCFG = [
    (48, 256, 4, 64, 384, 256),
    (256, 256, 2, 128, 384, 256),
    (256, 128, 1, 128, 256, 128),
]
TDEC_W = 128          # decode table row elems (bf16, 256B)
TB_W = 384            # padded trans_bias row (bf16)
DEC_TILE = 512
KCH = 2               # collective chunks per layer
PRO_G = 4             # prologue blocks per gather group

bf16 = ml_dtypes.bfloat16


def _wrap_idx(vals):
    """int16 index array for dma_gather: [128, n/16], wrapped over 16
    partitions and replicated across the 8 gpsimd cores."""
    n = len(vals)
    assert n % 16 == 0
    a = np.zeros((128, n // 16), np.int16)
    v = np.asarray(vals, np.int64)
    assert v.min() >= 0 and v.max() < 32768
    w = v.reshape(n // 16, 16).T.astype(np.int16)  # [16, n/16]
    for g in range(8):
        a[16 * g:16 * g + 16, :] = w
    return a


def _slotmajor(vals, fill, dtype):
    """[128, n/128] array with element (p, c) = vals[c*128+p]."""
    n = len(vals)
    assert n % 128 == 0
    return np.asarray(vals, np.float64).reshape(n // 128, 128).T.astype(dtype)


def prep(x, edge_index, edge_label_index, emb, W1, a_src1, a_dst1, b1,
         W2, a_src2, a_dst2, b2, W3, a_src3, a_dst3, b3,
         Wl1, bl1, Wl2, bl2, trans_bias):
    """Host-side (integer/index + weight-layout) preprocessing."""
    types = x[:, 0].astype(np.int64)

    # --- weight folds: RHS_l = [W_l | W_l@a_src per head | W_l@a_dst] ---
    def fold(W, a_s, a_d, H, D):
        cols_s = np.stack([W[:, h * D:(h + 1) * D] @ a_s[h] for h in range(H)], 1)
        cols_d = np.stack([W[:, h * D:(h + 1) * D] @ a_d[h] for h in range(H)], 1)
        return np.concatenate([W, cols_s, cols_d], 1).astype(bf16)
    RHS = [fold(W1, a_src1, a_dst1, 4, 64),
           fold(W2, a_src2, a_dst2, 2, 128),
           fold(W3, a_src3, a_dst3, 1, 128)]

    emb_pad = np.zeros((NTYPES, 128), bf16)
    emb_pad[:, :EMB] = emb.astype(bf16)
    TBpad = np.zeros((NTYPES, TB_W), bf16)
    TBpad[:, :NTYPES] = trans_bias.astype(bf16)

    # chunk-major row permutation: newrow[g] for gathered tables
    # (chunk0 rows of all cores first, then chunk1) so chunked AllGather
    # outputs are contiguous. T1 uses the same space (prologue inputs are
    # permuted host-side).
    CB0 = (NB // KCH) * 128
    C1R = SLICE - CB0
    gg = np.arange(NP, dtype=np.int64)
    cc_ = gg // SLICE
    rr = gg % SLICE
    newrow = np.where(rr < CB0, cc_ * CB0 + rr,
                      8 * CB0 + cc_ * C1R + (rr - CB0))
    orig = np.argsort(newrow)              # orig[nr] = g

    # full-table transposed features (bf16) + full type index (permuted)
    xT_full = np.zeros((32, NP), bf16)
    xT_full[:, :N] = x[:, 1:33].T.astype(bf16)
    xT_full = np.ascontiguousarray(xT_full[:, orig])
    types_full = np.zeros(NP, np.int64)
    types_full[:N] = types
    embt_full = _wrap_idx(types_full[orig])

    # --- edges: add self loops, sort by dst, bucket per core / block ---
    loops = np.arange(N, dtype=np.int64)
    src = np.concatenate([edge_index[0].astype(np.int64), loops])
    dst = np.concatenate([edge_index[1].astype(np.int64), loops])
    order = np.argsort(dst, kind="stable")
    src, dst = src[order], dst[order]
    src = newrow[src]                      # gather in permuted table space

    blk = dst // 128          # global block id (0..391)
    per = [[[None, None] for _ in range(NB)] for _ in range(8)]
    for c in range(8):
        for b in range(NB):
            gb = c * NB + b
            m = blk == gb
            s, d = src[m], dst[m]
            lo = s < HALF
            per[c][b][0] = (s[lo], d[lo])
            per[c][b][1] = (s[~lo] - HALF, d[~lo])
    CA = [max(1, max((len(per[c][b][0][0]) + 127) // 128 for c in range(8)))
          for b in range(NB)]
    CB = [max((len(per[c][b][1][0]) + 127) // 128 for c in range(8))
          for b in range(NB)]
    assert max(CA) <= 32 and max(CB) <= 32

    idxA, idxB, dsti, doff = [], [], [], []
    for c in range(8):
        la, lb, ld, lo = [], [], [], []
        for b in range(NB):
            for half, (cnt, acc) in (((0), (CA[b], la)), ((1), (CB[b], lb))):
                s, d = per[c][b][half]
                ns = cnt * 128
                sp = np.zeros(ns, np.int64)
                sp[:len(s)] = s
                acc.append(sp)
                dl = np.zeros(ns, np.int64)          # dst local to core slice
                dl[:len(d)] = d[:len(d)] - c * SLICE
                ld.append(dl)
                off = np.full(ns, 255, np.int64)     # 255 => padded slot
                off[:len(d)] = d[:len(d)] - (c * SLICE + b * 128)
                lo.append(off)
        idxA.append(_wrap_idx(np.concatenate(la)))
        idxB.append(_wrap_idx(np.concatenate(lb)))
        dsti.append(_wrap_idx(np.concatenate(ld)))
        doff.append(_slotmajor(np.concatenate(lo), 255, np.float32))

    # --- label edges: 4 groups by (ls-half, ld-half), padded per group ---
    ls0 = edge_label_index[0].astype(np.int64)
    ld0 = edge_label_index[1].astype(np.int64)
    ls = newrow[ls0]
    ld_ = newrow[ld0]
    elpc = (EL + 7) // 8
    groups_sz = np.zeros((8, 4), np.int64)
    per_dec = [[None] * 4 for _ in range(8)]
    for c in range(8):
        lo_, hi_ = c * elpc, min((c + 1) * elpc, EL)
        eidx = np.arange(lo_, hi_)
        g = (ls[eidx] >= HALF).astype(np.int64) * 2 + (ld_[eidx] >= HALF)
        for gi in range(4):
            per_dec[c][gi] = eidx[g == gi]
            groups_sz[c, gi] = len(per_dec[c][gi])
    GSZ = [int(-(-groups_sz[:, gi].max() // DEC_TILE) * DEC_TILE)
           for gi in range(4)]
    SL = sum(GSZ)
    lsw, ldw, tlsw, tldw, slotmap = [], [], [], [], []
    for c in range(8):
        a_ls = np.zeros(SL, np.int64)
        a_ld = np.zeros(SL, np.int64)
        a_tls = np.zeros(SL, np.int64)
        a_tld = np.zeros(SL, np.int64)
        smap = np.full(SL, -1, np.int64)
        pos = 0
        for gi in range(4):
            e = per_dec[c][gi]
            n = len(e)
            a_ls[pos:pos + n] = ls[e] - (HALF if gi >= 2 else 0)
            a_ld[pos:pos + n] = ld_[e] - (HALF if gi % 2 else 0)
            a_tls[pos:pos + n] = types[np.minimum(ls0[e], N - 1)]
            a_tld[pos:pos + n] = types[np.minimum(ld0[e], N - 1)]
            smap[pos:pos + n] = e
            pos += GSZ[gi]
        lsw.append(_wrap_idx(a_ls))
        ldw.append(_wrap_idx(a_ld))
        tlsw.append(_wrap_idx(a_tls))
        tldw.append(_slotmajor(a_tld, 0, np.float32))
        slotmap.append(smap)

    iota128 = np.tile(np.arange(128, dtype=np.float32)[None, :], (128, 1))
    iota384 = np.tile(np.arange(TB_W, dtype=np.float32)[None, :], (128, 1))
    identB = np.eye(128, dtype=bf16)

    in_maps = []
    for c in range(8):
        n0 = c * SLICE
        xT_own = np.zeros((32, SLICE), bf16)
        xT_own[:] = xT_full[:, n0:n0 + SLICE]
        embt_own = _wrap_idx(types_full[n0:n0 + SLICE])
        in_maps.append(dict(
            xT_full=xT_full, embt_full=embt_full,
            xT_own=xT_own, embt_own=embt_own,
            emb_pad=emb_pad,
            RHS1e=RHS[0][0:16], RHS1x=RHS[0][16:48],
            RHS2=RHS[1], RHS3=RHS[2],
            idxA=idxA[c], idxB=idxB[c], dsti=dsti[c], doff=doff[c],
            ls_idx=lsw[c], ld_idx=ldw[c], tls_idx=tlsw[c], tld=tldw[c],
            iota128=iota128, iota384=iota384, identB=identB,
            TBpad=TBpad,
            Wl1a=Wl1[:128].astype(bf16), Wl1b=Wl1[128:].astype(bf16),
            Wl2=Wl2.astype(bf16), bl1=bl1.reshape(64, 1).astype(np.float32),
        ))
    cfg = dict(CA=CA, CB=CB, GSZ=GSZ, SL=SL, newrow=newrow,
               SA=sum(CA) * 128, SB=sum(CB) * 128,
               ST=(sum(CA) + sum(CB)) * 128,
               bl2=float(np.asarray(bl2).reshape(-1)[0]),
               b=[np.asarray(b1), np.asarray(b2), np.asarray(b3)],
               slotmap=slotmap)
    return in_maps, cfg


# ---------------------------------------------------------------- golden ---
def golden(in_maps, cfg):
    """numpy mirror of the device algorithm (layout/precision-accurate)."""
    CA, CB = cfg["CA"], cfg["CB"]

    def f(a):
        return np.asarray(a, np.float32)

    out_all = []
    # T1: full table on every core (identical), bf16
    im0 = in_maps[0]
    RHS1 = np.concatenate([f(im0["RHS1e"]), f(im0["RHS1x"])], axis=0)
    # rowmap[c] = permuted-table rows of core c's local rows 0..SLICE
    nr = cfg["newrow"]
    rowmap = [nr[c * SLICE:(c + 1) * SLICE] for c in range(8)]
    embT = f(im0["emb_pad"])[_unwrap(im0["embt_full"], NP)][:, :EMB]
    x0_full = np.concatenate([embT, f(im0["xT_full"]).T], axis=1)  # [NP,48]
    T = np.zeros((NP, 384), np.float32)
    T[:, :264] = (x0_full @ RHS1).astype(bf16)   # already in permuted order
    # per-core ald tables for L1 (cols 0:8 = als|ald)
    ald_t = []
    for c in range(8):
        im = in_maps[c]
        embo = f(im["emb_pad"])[_unwrap(im["embt_own"], SLICE)][:, :EMB]
        x0 = np.concatenate([embo, f(im["xT_own"]).T], axis=1)
        ald_t.append((x0 @ RHS1[:, 256:264]).astype(bf16).astype(np.float32))

    for li, (Din, HD, H, D, STRIDE, OFF) in enumerate(LCFG):
        PREV = []
        nald_t = []
        Tn = np.zeros((NP, 384), np.float32)
        for c in range(8):
            im = in_maps[c]
            ia = _unwrap(im["idxA"], cfg["SA"])
            ib = _unwrap(im["idxB"], cfg["SB"])
            idt = _unwrap(im["dsti"], cfg["ST"])
            dof = im["doff"].T.reshape(-1)
            xl = np.zeros((SLICE, HD), np.float32)
            pa = pb = pt = 0
            for b in range(NB):
                sA, sB = CA[b] * 128, CB[b] * 128
                gidx = np.concatenate([ia[pa:pa + sA],
                                       ib[pb:pb + sB] + HALF])
                pa += sA; pb += sB
                nsl = sA + sB
                G = T[gidx, :]                     # [nsl, STRIDE]
                GD = ald_t[c][idt[pt:pt + nsl]]
                off = dof[pt:pt + nsl]
                pt += nsl
                S = (off[:, None] == np.arange(128)[None, :]).astype(np.float32)
                als = G[:, OFF:OFF + H]
                ald = GD[:, H:2 * H]
                lg = als + ald
                lg = np.where(lg > 0, lg, 0.2 * lg)
                e = np.exp(lg).astype(bf16).astype(np.float32)
                eex = np.repeat(e, D, axis=1)      # [nsl, HD]
                msg = (G[:, :HD].astype(bf16).astype(np.float32) * eex
                       ).astype(bf16).astype(np.float32)
                num = S.T @ msg                    # [128, HD]
                den = S.T @ e                      # [128, H]
                r = 1.0 / (den + 1e-16)
                xb = (num.reshape(128, H, D) * np.repeat(r, D, 1).reshape(128, H, D)
                      ).reshape(128, HD).astype(bf16).astype(np.float32)
                if li < 2:
                    xb = (np.maximum(xb, 0)
                          + np.exp(np.minimum(xb, 0)).astype(bf16).astype(np.float32)
                          - 1).astype(bf16).astype(np.float32)
                xl[b * 128:(b + 1) * 128] = xb
            PREV.append(xl)
            if li < 2:
                NDin, NHD, NH, ND, NSTRIDE, _ = LCFG[li + 1]
                NW = NHD + 2 * NH
                hrow = (xl.astype(bf16).astype(np.float32)
                        @ f(im[f"RHS{li + 2}"])).astype(bf16).astype(np.float32)
                Tn[rowmap[c], :NW] = hrow
                nald_t.append(hrow[:, NHD:NHD + 2 * NH])
        if li < 2:
            T = Tn
            ald_t = nald_t
    # decode
    TD = np.zeros((NP, TDEC_W), np.float32)
    for c in range(8):
        TD[rowmap[c]] = PREV[c].astype(bf16)
    scores = []
    for c in range(8):
        im = in_maps[c]
        lsv = _unwrap(im["ls_idx"], cfg["SL"])
        ldv = _unwrap(im["ld_idx"], cfg["SL"])
        tlsv = _unwrap(im["tls_idx"], cfg["SL"])
        tldv = im["tld"].T.reshape(-1)
        base_ls = np.zeros(cfg["SL"], np.int64)
        base_ld = np.zeros(cfg["SL"], np.int64)
        pos = 0
        for gi in range(4):
            base_ls[pos:pos + cfg["GSZ"][gi]] = HALF if gi >= 2 else 0
            base_ld[pos:pos + cfg["GSZ"][gi]] = HALF if gi % 2 else 0
            pos += cfg["GSZ"][gi]
        zl = TD[lsv + base_ls]
        zr = TD[ldv + base_ld]
        W1a = f(in_maps[c]["Wl1a"])
        W1b = f(in_maps[c]["Wl1b"])
        h = np.maximum(zl @ W1a + zr @ W1b + in_maps[c]["bl1"].T, 0
                       ).astype(bf16).astype(np.float32)
        base = h @ f(in_maps[c]["Wl2"]) + cfg["bl2"]
        TBg = f(in_maps[c]["TBpad"])[tlsv]
        oh = (tldv[:, None] == np.arange(TB_W)[None, :])
        bias = (TBg * oh).sum(1)
        scores.append(base[:, 0] + bias)
    out = np.zeros((EL, 1), np.float32)
    for c in range(8):
        m = cfg["slotmap"][c] >= 0
        out[cfg["slotmap"][c][m], 0] = scores[c][m]
    return out


def _unwrap(w, n):
    return w[:16, :].T.reshape(-1)[:n].astype(np.int64)


# ----------------------------------------------------------------- device ---
def build(cfg, cut='full', exp='orig', kch=KCH, nq=4, feat='all',
          gbufs=2, prog=PRO_G, gdw=2):  # cuts: noop,prob,pro,l1..l3,full
    import concourse.bacc as bacc
    import concourse.mybir as mybir
    from concourse.tile import TileContext
    dt = mybir.dt
    F = mybir.ActivationFunctionType
    A = mybir.AluOpType
    CA, CB, SL = cfg["CA"], cfg["CB"], cfg["SL"]
    SA, SB, ST = cfg["SA"], cfg["SB"], cfg["ST"]

    nc = bacc.Bacc(num_devices=8, dynamic_dma_scratch_size=57344,
                   num_swdge_queues=nq)
    GMAX = 8  # max 128-chunks per dma_gather call (1024-desc HW cap)

    qctr = [0]

    def nextq():
        qctr[0] += 1
        return qctr[0] % nq

    def gat(out_ap, in_ap, idx_tile, col0, nchunk, elem, q=None, **kw):
        for s0 in range(0, nchunk, GMAX):
            s1 = min(s0 + GMAX, nchunk)
            qn = nextq() if q is None else q
            nc.gpsimd.dma_gather(
                out_ap[:, s0:s1, :], in_ap,
                idx_tile[:, col0 + s0 * 8: col0 + s1 * 8],
                (s1 - s0) * 128, (s1 - s0) * 128, elem, queue_num=qn, **kw)
    inp = {}
    for name, shape, d in [
        ("xT_full", [32, NP], dt.bfloat16),
        ("embt_full", [128, NP // 16], dt.int16),
        ("xT_own", [32, SLICE], dt.bfloat16),
        ("embt_own", [128, SLICE // 16], dt.int16),
        ("emb_pad", [NTYPES, 128], dt.bfloat16),
        ("RHS1e", [16, 264], dt.bfloat16),
        ("RHS1x", [32, 264], dt.bfloat16),
        ("RHS2", [256, 260], dt.bfloat16),
        ("RHS3", [256, 130], dt.bfloat16),
        ("idxA", [128, SA // 16], dt.int16),
        ("idxB", [128, SB // 16], dt.int16),
        ("dsti", [128, ST // 16], dt.int16),
        ("doff", [128, ST // 128], dt.float32),
        ("ls_idx", [128, SL // 16], dt.int16),
        ("ld_idx", [128, SL // 16], dt.int16),
        ("tls_idx", [128, SL // 16], dt.int16),
        ("tld", [128, SL // 128], dt.float32),
        ("iota128", [128, 128], dt.float32),
        ("iota384", [128, TB_W], dt.float32),
        ("identB", [128, 128], dt.bfloat16),
        ("TBpad", [NTYPES, TB_W], dt.bfloat16),
        ("Wl1a", [128, 64], dt.bfloat16),
        ("Wl1b", [128, 64], dt.bfloat16),
        ("Wl2", [64, 1], dt.bfloat16),
        ("bl1", [64, 1], dt.float32),
    ]:
        inp[name] = nc.dram_tensor(name, shape, d, kind="ExternalInput")
    score_out = nc.dram_tensor("score", [SL, 1], dt.float32, kind="ExternalOutput")

    # chunked slice tensors (distinct per collective chunk, no false WAR deps)
    cb0 = (NB // KCH) * 128                      # rows in chunk 0
    crows = [cb0, SLICE - cb0]
    sl_t = {}
    for l, w in ((1, 384), (2, 256)):
        sl_t[l] = [nc.dram_tensor(f"slice{l}_{k}", [crows[k], w], dt.bfloat16,
                                  kind="Internal") for k in range(KCH)]
    sl_d = [nc.dram_tensor(f"sliceD_{k}", [crows[k], TDEC_W], dt.bfloat16,
                           kind="Internal") for k in range(KCH)]
    T1loc = nc.dram_tensor("T1loc", [NP, 384], dt.bfloat16, kind="Internal")
    T_t = {l: nc.dram_tensor(f"T{l}", [NP, w], dt.bfloat16, kind="Internal",
                             addr_space="Shared")
           for l, w in ((1, 384), (2, 256))}
    T_d = nc.dram_tensor("TD", [NP, TDEC_W], dt.bfloat16,
                         kind="Internal", addr_space="Shared")
    sl_ald = [nc.dram_tensor(f"ald{l}", [SLICE, 128], dt.bfloat16,
                             kind="Internal") for l in range(3)]

    with TileContext(nc, num_cores=8) as tc:
        with tc.tile_pool(name="const", bufs=1) as cpool, \
             tc.tile_pool(name="work", bufs=2) as wpool, \
             tc.tile_pool(name="gat", bufs=gbufs) as gpool, \
             tc.tile_pool(name="psum", bufs=2, space="PSUM") as ppool, \
             tc.tile_pool(name="psum1", bufs=1, space="PSUM") as ppool1:
            # ---- resident constants / indices ----
            def load(name, shape, d):
                t = cpool.tile(shape, d, tag=name)
                nc.sync.dma_start(t[:], inp[name][:])
                return t
            idxA = load("idxA", [128, SA // 16], dt.int16)
            idxB = load("idxB", [128, SB // 16], dt.int16)
            dsti = load("dsti", [128, ST // 16], dt.int16)
            doff = load("doff", [128, ST // 128], dt.float32)
            iota = load("iota128", [128, 128], dt.float32)
            identB = load("identB", [128, 128], dt.bfloat16)
            RHS1e = load("RHS1e", [16, 264], dt.bfloat16)
            RHS1x = load("RHS1x", [32, 264], dt.bfloat16)
            RHSs = [None]
            for l, w in ((2, 260), (3, 130)):
                t = cpool.tile([128, 2, w], dt.bfloat16, tag=f"RHS{l}")
                nc.sync.dma_start(
                    t[:], inp[f"RHS{l}"][:].rearrange("(k p) w -> p k w", p=128))
                RHSs.append(t)
            embt_full = load("embt_full", [128, NP // 16], dt.int16)
            embt_own = load("embt_own", [128, SLICE // 16], dt.int16)

            # ---- prologue (b): own-slice ald table for L1 (cols 0:8) ----
            GN = prog * 128
            for g0 in (range(0, NB, prog) if cut != 'noop' else []):
                g1 = min(g0 + prog, NB)
                nblk = g1 - g0
                embT = wpool.tile([128, 1, GN], dt.bfloat16, tag="embT")
                nc.gpsimd.dma_gather(
                    embT[:, :, 0:nblk * 128], inp["emb_pad"][:],
                    embt_own[:, g0 * 8:g1 * 8], nblk * 128, nblk * 128, 128,
                    transpose=True, queue_num=nextq())
                xTg = wpool.tile([32, GN], dt.bfloat16, tag="xTg")
                nc.sync.dma_start(xTg[:, 0:nblk * 128],
                                  inp["xT_own"][:, g0 * 128:g1 * 128])
                for j in range(nblk):
                    pn8 = ppool1.tile([128, 8], dt.float32, tag="pn")
                    nc.tensor.matmul(pn8[:], embT[0:16, 0, j * 128:(j + 1) * 128],
                                     RHS1e[:, 256:264], start=True, stop=False)
                    nc.tensor.matmul(pn8[:], xTg[:, j * 128:(j + 1) * 128],
                                     RHS1x[:, 256:264], start=False, stop=True)
                    b = g0 + j
                    r8 = wpool.tile([128, 8], dt.bfloat16, tag="r8")
                    nc.scalar.copy(r8[:], pn8[:])
                    nc.sync.dma_start(sl_ald[0][b * 128:(b + 1) * 128, 0:8],
                                      r8[:])

            # ---- trans-bias (input-only): compute during prologue ----
            iota384 = load("iota384", [128, TB_W], dt.float32)
            tlsi = load("tls_idx", [128, SL // 16], dt.int16)
            tld = load("tld", [128, SL // 128], dt.float32)
            biassb = cpool.tile([128, SL // 128], dt.float32, tag="biassb")
            if cut == 'full':
                pos0 = 0
                for gi in range(4):
                    for t0 in range(pos0, pos0 + cfg["GSZ"][gi], DEC_TILE):
                        TBg = wpool.tile([128, 4, TB_W], dt.bfloat16, tag="TBg")
                        nc.gpsimd.dma_gather(
                            TBg[:], inp["TBpad"][:],
                            tlsi[:, t0 // 16:(t0 + DEC_TILE) // 16],
                            DEC_TILE, DEC_TILE, TB_W, queue_num=nextq())
                        oh = wpool.tile([128, 4, TB_W], dt.bfloat16, tag="oh")
                        nc.vector.tensor_tensor(
                            oh[:],
                            tld[:, t0 // 128: t0 // 128 + 4].unsqueeze(-1)
                                .broadcast_to([128, 4, TB_W]),
                            iota384[:].unsqueeze(1).broadcast_to([128, 4, TB_W]),
                            A.is_equal)
                        nc.vector.tensor_tensor(oh[:], TBg[:], oh[:], A.mult)
                        nc.vector.tensor_reduce(
                            biassb[:, t0 // 128:t0 // 128 + 4], oh[:],
                            mybir.AxisListType.X, A.add)
                    pos0 += cfg["GSZ"][gi]

            # ---- prologue (a): full T1 on every core ----
            for g0 in (range(0, NBF, prog) if cut not in ('prob', 'noop') else []):
                g1 = min(g0 + prog, NBF)
                nblk = g1 - g0
                embT = wpool.tile([128, 1, GN], dt.bfloat16, tag="embT")
                nc.gpsimd.dma_gather(
                    embT[:, :, 0:nblk * 128], inp["emb_pad"][:],
                    embt_full[:, g0 * 8:g1 * 8], nblk * 128, nblk * 128, 128,
                    transpose=True, queue_num=nextq())
                xTg = wpool.tile([32, GN], dt.bfloat16, tag="xTg")
                nc.sync.dma_start(xTg[:, 0:nblk * 128],
                                  inp["xT_full"][:, g0 * 128:g1 * 128])
                for j in range(nblk):
                    pn = ppool1.tile([128, 264], dt.float32, tag="pn")
                    nc.tensor.matmul(pn[:], embT[0:16, 0, j * 128:(j + 1) * 128],
                                     RHS1e[:], start=True, stop=False)
                    nc.tensor.matmul(pn[:], xTg[:, j * 128:(j + 1) * 128],
                                     RHS1x[:], start=False, stop=True)
                    row = wpool.tile([128, 264], dt.bfloat16, tag="rowT1")
                    nc.scalar.copy(row[:], pn[:])
                    b = g0 + j
                    nc.sync.dma_start(T1loc[b * 128:(b + 1) * 128, 0:264], row[:])

            # ---- three GAT layers ----
            NL = {'noop': 0, 'prob': 0, 'pro': 0, 'l1': 1, 'l2': 2, 'l3': 3, 'full': 3}[cut]
            for li, (Din, HD, H, D, STRIDE, OFF) in enumerate(LCFG[:NL]):
                RW = HD + H
                T_src = T1loc if li == 0 else T_t[li]
                pa = pb = pt_ = 0
                gdt = {}
                for b in range(NB):
                    cA, cB = CA[b], CB[b]
                    C = cA + cB
                    if b % gdw == 0:
                        wblocks = range(b, min(b + gdw, NB))
                        CW = sum(CA[j] + CB[j] for j in wblocks)
                        GDw = gpool.tile([128, CW, 128], dt.bfloat16, tag="GD")
                        gat(GDw, sl_ald[li][:, :], dsti[:], pt_ // 16, CW, 128)
                        off = 0
                        for j in wblocks:
                            gdt[j] = (GDw, off)
                            off += CA[j] + CB[j]
                    GE = ((STRIDE * 2 // 3 if STRIDE == 384 else STRIDE // 2)
                          if feat == 'g512' else STRIDE)
                    OFFg = OFF if feat != 'g512' else GE - H
                    G = gpool.tile([128, C, GE], dt.bfloat16, tag="G")
                    gat(G, T_src[:, 0:GE], idxA[:], pa // 16, cA, GE,
                        elem_step=STRIDE)
                    if cB:
                        gat(G[:, cA:C, :].rearrange("p c e -> p c e"),
                            T_src[HALF:, 0:GE], idxB[:], pb // 16, cB, GE,
                            elem_step=STRIDE)
                    S = wpool.tile([128, C, 128], dt.bfloat16, tag="S")
                    nc.vector.tensor_tensor(
                        S[:],
                        doff[:, pt_ // 128: pt_ // 128 + C].unsqueeze(-1)
                            .broadcast_to([128, C, 128]),
                        iota[:].unsqueeze(1).broadcast_to([128, C, 128]),
                        A.is_equal)
                    GD = gpool.tile([128, C, 128], dt.bfloat16, tag="GD")
                    gat(GD, sl_ald[li][:, :], dsti[:], pt_ // 16, C, 128)
                    lg = wpool.tile([128, C, H], dt.float32, tag="lg")
                    nc.vector.tensor_tensor(
                        lg[:], G[:, :, OFFg:OFFg + H], GD[:, :, H:2 * H],
                        A.add)
                    lg2 = wpool.tile([128, C, H], dt.float32, tag="lg2")
                    nc.vector.tensor_scalar_mul(lg2[:], lg[:], 0.2)
                    nc.vector.tensor_tensor(lg[:], lg[:], lg2[:], A.max)
                    RT = wpool.tile([128, C, RW], dt.bfloat16, tag="RT")
                    nc.scalar.activation(RT[:, :, HD:HD + H], lg[:], F.Exp)
                    if exp == 'bcast':
                        nc.scalar.activation(
                            RT[:, :, 0:HD].rearrange("p c (h d) -> p c h d", h=H),
                            lg[:].unsqueeze(-1).broadcast_to([128, C, H, D]),
                            F.Exp)
                        nc.vector.tensor_tensor(
                            RT[:, :, 0:HD], RT[:, :, 0:HD], G[:, :, 0:HD], A.mult)
                    else:
                        nc.vector.tensor_tensor(
                            RT[:, :, 0:HD].rearrange("p c (h d) -> p c h d", h=H),
                            G[:, :, 0:HD].rearrange("p c (h d) -> p c h d", h=H),
                            RT[:, :, HD:HD + H].unsqueeze(-1)
                                .broadcast_to([128, C, H, D]),
                            A.mult)
                    pe = ppool.tile([128, RW], dt.float32, tag="pe")
                    NMM = 1 if feat == 'nomm' else C
                    for ch in range(NMM):
                        nc.tensor.matmul(pe[:, 0:RW], S[:, ch, :], RT[:, ch, :],
                                         start=(ch == 0), stop=(ch == NMM - 1))
                    pa += cA * 128
                    pb += cB * 128
                    pt_ += C * 128
                    # ---- finalize + node phase ----
                    den = wpool.tile([128, H], dt.float32, tag="den")
                    nc.vector.tensor_scalar_add(den[:], pe[:, HD:HD + H], 1e-16)
                    rec = wpool.tile([128, H], dt.float32, tag="rec")
                    nc.vector.reciprocal(rec[:], den[:])
                    xo = wpool.tile([128, HD], dt.bfloat16, tag="xo")
                    nc.vector.tensor_tensor(
                        xo[:].rearrange("p (h d) -> p h d", h=H),
                        pe[:, 0:HD].rearrange("p (h d) -> p h d", h=H),
                        rec[:].unsqueeze(-1).broadcast_to([128, H, D]),
                        A.mult)
                    kc_ = 0 if b < NB // KCH else 1
                    rb = b - (0 if kc_ == 0 else NB // KCH)
                    if li < 2:
                        m = wpool.tile([128, HD], dt.bfloat16, tag="melu")
                        nc.vector.tensor_scalar_min(m[:], xo[:], 0.0)
                        e1 = wpool.tile([128, HD], dt.bfloat16, tag="e1")
                        nc.scalar.activation(e1[:], m[:], F.Exp)
                        nc.vector.tensor_scalar_max(xo[:], xo[:], 0.0)
                        nc.vector.tensor_tensor(xo[:], xo[:], e1[:], A.add)
                        nc.vector.tensor_scalar_add(xo[:], xo[:], -1.0)
                        NDin, NHD, NH, ND, NSTRIDE, _ = LCFG[li + 1]
                        NW = NHD + 2 * NH
                        xT = wpool.tile([128, 2, 128], dt.bfloat16, tag="xT")
                        for kc in range(2):
                            ptp = ppool1.tile([128, 128], dt.bfloat16, tag="pt")
                            nc.tensor.transpose(
                                ptp[:], xo[:, kc * 128:(kc + 1) * 128], identB[:])
                            nc.scalar.copy(xT[:, kc, :], ptp[:])
                        pn = ppool1.tile([128, 264], dt.float32, tag="pn")
                        for kc in range(2):
                            nc.tensor.matmul(pn[:, 0:NW], xT[:, kc, :],
                                             RHSs[li + 1][:, kc, :],
                                             start=(kc == 0), stop=(kc == 1))
                        row = wpool.tile([128, NW], dt.bfloat16, tag="row")
                        nc.scalar.copy(row[:], pn[:, 0:NW])
                        nc.sync.dma_start(
                            sl_t[li + 1][kc_][rb * 128:(rb + 1) * 128, 0:NW],
                            row[:])
                        nc.sync.dma_start(
                            sl_ald[li + 1][b * 128:(b + 1) * 128, 0:2 * NH],
                            row[:, NHD:NHD + 2 * NH])
                    else:
                        nc.sync.dma_start(
                            sl_d[kc_][rb * 128:(rb + 1) * 128, :], xo[:])
                    # chunked collective as soon as a chunk's blocks are done
                    cpoints = ([NB // KCH - 1, NB - 1] if kch == 2
                               else [NB - 1, NB - 1])
                    for k in [i for i, cp in enumerate(cpoints) if cp == b]:
                        o0 = 0 if k == 0 else 8 * cb0
                        o1 = o0 + 8 * crows[k]
                        if li < 2:
                            nc.gpsimd.collective_compute(
                                "AllGather", mybir.AluOpType.bypass,
                                ins=[sl_t[li + 1][k][:]],
                                outs=[T_t[li + 1][o0:o1, :]],
                                replica_groups=[list(range(8))])
                        else:
                            nc.gpsimd.collective_compute(
                                "AllGather", mybir.AluOpType.bypass,
                                ins=[sl_d[k][:]],
                                outs=[T_d[o0:o1, :]],
                                replica_groups=[list(range(8))])

            # ---- decoder ----
            DECODE = cut == 'full'
            lsi = load("ls_idx", [128, SL // 16], dt.int16)
            ldi = load("ld_idx", [128, SL // 16], dt.int16)
            W1a = load("Wl1a", [128, 64], dt.bfloat16)
            W1b = load("Wl1b", [128, 64], dt.bfloat16)
            W2d = load("Wl2", [64, 1], dt.bfloat16)
            bl1 = load("bl1", [64, 1], dt.float32)
            score_sb = cpool.tile([128, SL // 128], dt.float32, tag="score")
            if not DECODE:
                nc.vector.memset(score_sb[:], 0)
            pos = 0
            for gi in range(4 if DECODE else 0):
                gls, gld = (HALF if gi >= 2 else 0), (HALF if gi % 2 else 0)
                for t0 in range(pos, pos + cfg["GSZ"][gi], DEC_TILE):
                    zl = wpool.tile([128, 1, DEC_TILE], dt.bfloat16, tag="zl")
                    nc.gpsimd.dma_gather(
                        zl[:], T_d[gls:, :], lsi[:, t0 // 16:(t0 + DEC_TILE) // 16],
                        DEC_TILE, DEC_TILE, TDEC_W, transpose=True,
                        queue_num=nextq())
                    zr = wpool.tile([128, 1, DEC_TILE], dt.bfloat16, tag="zr")
                    nc.gpsimd.dma_gather(
                        zr[:], T_d[gld:, :], ldi[:, t0 // 16:(t0 + DEC_TILE) // 16],
                        DEC_TILE, DEC_TILE, TDEC_W, transpose=True,
                        queue_num=nextq())
                    ph = ppool.tile([64, DEC_TILE], dt.float32, tag="pe")
                    nc.tensor.matmul(ph[:], W1a[:], zl[:, 0, :], start=True, stop=False)
                    nc.tensor.matmul(ph[:], W1b[:], zr[:, 0, :], start=False, stop=True)
                    hd = wpool.tile([64, DEC_TILE], dt.bfloat16, tag="hd")
                    nc.scalar.activation(hd[:], ph[:], F.Relu, bias=bl1[:])
                    for sub in range(4):
                        pss = ppool1.tile([128, 1], dt.float32, tag="pn")
                        nc.tensor.matmul(pss[:], hd[:, sub * 128:(sub + 1) * 128],
                                         W2d[:], start=True, stop=True)
                        col = t0 // 128 + sub
                        nc.vector.tensor_tensor(
                            score_sb[:, col:col + 1], pss[:],
                            biassb[:, col:col + 1], A.add)
                pos += cfg["GSZ"][gi]
            if cfg["bl2"] != 0.0:
                nc.vector.tensor_scalar_add(score_sb[:], score_sb[:], cfg["bl2"])
            nc.sync.dma_start(
                score_out[:].rearrange("(c p) o -> p (c o)", p=128), score_sb[:])
    nc.finalize()
    return nc


def kernel(**inputs):
    inputs = {k: np.asarray(v) for k, v in inputs.items()}
    in_maps, cfg = prep(**inputs)
    nc = build(cfg)
    from concourse.bass_utils import run_bass_kernel_spmd
    res = run_bass_kernel_spmd(nc, in_maps, core_ids=list(range(8)))
    out = np.zeros((EL, 1), np.float32)
    for c in range(8):
        sc = res.results[c]["score"][:, 0]
        m = cfg["slotmap"][c] >= 0
        out[cfg["slotmap"][c][m], 0] = sc[m]
    return out


# revision 44
# speedup vs baseline: 1.3189x; 1.3189x over previous
"""Trainium2 Bass kernel for nn_DAGLinkPredictor (3-layer GAT + edge decoder).

Sharding: dst-node-sharded GAT across 8 cores. Edges (incl self-loops) are
sorted by dst and grouped into per-core 128-node dst blocks. Per block:
  - dma_gather pulls [h | als] rows (bf16) for edge sources from the node
    table T_l (two gathers: src < 32768 and >= 32768, int16 gather indices),
  - dst-side [als|ald] comes from compact local DRAM tables (256B rows)
    via dma_gather (an S^T@ald PE-matmul alternative measured 2x slower on
    real HW: per-chunk DVE->PE->Act->PE semaphore chains),
  - a one-hot scatter matrix S (VectorE is_equal) scatter-adds messages into
    PSUM via TensorE,
  - softmax is denominator-style: out[d] = sum_e exp(lrelu(als+ald)) * h[src]
    / sum_e exp(...) (exactly segment-softmax; logits are O(1) so no max).
T1 is computed fully on every core from the replicated input (no collective,
prologue inputs pre-permuted to the chunk-major table layout). T2/T3/TD
slices are AllGathered in 2 contiguous chunks overlapped with the block
loop. Gathers round-robin over 4 SWDGE queues (the serialized per-call
completion wait was the dominant real-HW cost). Node-phase matmuls are
bf16.
Decoder: transpose-gathers of z rows + dense matmuls, trans_bias via
gathered rows x one-hot reduce.
"""
import numpy as np
import ml_dtypes

N = 50000
NP = 50176            # padded nodes: 8 * 6272
SLICE = NP // 8       # 6272 nodes per core
NB = SLICE // 128     # 49 blocks per core
NBF = NP // 128       # 392 blocks in the full table
E = 800000
EL = 100000
HALF = 32768          # int16 gather index limit
NTYPES = 311
EMB = 16
COMB = 48

# layer configs: (Din, HD, H, D, src_elem(row stride), dst_off_elems)
LCFG = [
    (48, 256, 4, 64, 384, 256),
    (256, 256, 2, 128, 384, 256),
    (256, 128, 1, 128, 256, 128),
]
TDEC_W = 128          # decode table row elems (bf16, 256B)
TB_W = 384            # padded trans_bias row (bf16)
DEC_TILE = 512
KCH = 2               # collective chunks per layer
PRO_G = 4             # prologue blocks per gather group

bf16 = ml_dtypes.bfloat16


def _wrap_idx(vals):
    """int16 index array for dma_gather: [128, n/16], wrapped over 16
    partitions and replicated across the 8 gpsimd cores."""
    n = len(vals)
    assert n % 16 == 0
    a = np.zeros((128, n // 16), np.int16)
    v = np.asarray(vals, np.int64)
    assert v.min() >= 0 and v.max() < 32768
    w = v.reshape(n // 16, 16).T.astype(np.int16)  # [16, n/16]
    for g in range(8):
        a[16 * g:16 * g + 16, :] = w
    return a


def _slotmajor(vals, fill, dtype):
    """[128, n/128] array with element (p, c) = vals[c*128+p]."""
    n = len(vals)
    assert n % 128 == 0
    return np.asarray(vals, np.float64).reshape(n // 128, 128).T.astype(dtype)


def prep(x, edge_index, edge_label_index, emb, W1, a_src1, a_dst1, b1,
         W2, a_src2, a_dst2, b2, W3, a_src3, a_dst3, b3,
         Wl1, bl1, Wl2, bl2, trans_bias):
    """Host-side (integer/index + weight-layout) preprocessing."""
    types = x[:, 0].astype(np.int64)

    # --- weight folds: RHS_l = [W_l | W_l@a_src per head | W_l@a_dst] ---
    def fold(W, a_s, a_d, H, D):
        cols_s = np.stack([W[:, h * D:(h + 1) * D] @ a_s[h] for h in range(H)], 1)
        cols_d = np.stack([W[:, h * D:(h + 1) * D] @ a_d[h] for h in range(H)], 1)
        return np.concatenate([W, cols_s, cols_d], 1).astype(bf16)
    RHS = [fold(W1, a_src1, a_dst1, 4, 64),
           fold(W2, a_src2, a_dst2, 2, 128),
           fold(W3, a_src3, a_dst3, 1, 128)]

    emb_pad = np.zeros((NTYPES, 128), bf16)
    emb_pad[:, :EMB] = emb.astype(bf16)
    TBpad = np.zeros((NTYPES, TB_W), bf16)
    TBpad[:, :NTYPES] = trans_bias.astype(bf16)

    # chunk-major row permutation: newrow[g] for gathered tables
    # (chunk0 rows of all cores first, then chunk1) so chunked AllGather
    # outputs are contiguous. T1 uses the same space (prologue inputs are
    # permuted host-side).
    CB0 = (NB // KCH) * 128
    C1R = SLICE - CB0
    gg = np.arange(NP, dtype=np.int64)
    cc_ = gg // SLICE
    rr = gg % SLICE
    newrow = np.where(rr < CB0, cc_ * CB0 + rr,
                      8 * CB0 + cc_ * C1R + (rr - CB0))
    orig = np.argsort(newrow)              # orig[nr] = g

    # full-table transposed features (bf16) + full type index (permuted)
    xT_full = np.zeros((32, NP), bf16)
    xT_full[:, :N] = x[:, 1:33].T.astype(bf16)
    xT_full = np.ascontiguousarray(xT_full[:, orig])
    types_full = np.zeros(NP, np.int64)
    types_full[:N] = types
    embt_full = _wrap_idx(types_full[orig])

    # --- edges: add self loops, sort by dst, bucket per core / block ---
    loops = np.arange(N, dtype=np.int64)
    src = np.concatenate([edge_index[0].astype(np.int64), loops])
    dst = np.concatenate([edge_index[1].astype(np.int64), loops])
    order = np.argsort(dst, kind="stable")
    src, dst = src[order], dst[order]
    src = newrow[src]                      # gather in permuted table space

    blk = dst // 128          # global block id (0..391)
    per = [[[None, None] for _ in range(NB)] for _ in range(8)]
    for c in range(8):
        for b in range(NB):
            gb = c * NB + b
            m = blk == gb
            s, d = src[m], dst[m]
            lo = s < HALF
            per[c][b][0] = (s[lo], d[lo])
            per[c][b][1] = (s[~lo] - HALF, d[~lo])
    CA = [max(1, max((len(per[c][b][0][0]) + 127) // 128 for c in range(8)))
          for b in range(NB)]
    CB = [max((len(per[c][b][1][0]) + 127) // 128 for c in range(8))
          for b in range(NB)]
    assert max(CA) <= 32 and max(CB) <= 32

    idxA, idxB, dsti, doff = [], [], [], []
    for c in range(8):
        la, lb, ld, lo = [], [], [], []
        for b in range(NB):
            for half, (cnt, acc) in (((0), (CA[b], la)), ((1), (CB[b], lb))):
                s, d = per[c][b][half]
                ns = cnt * 128
                sp = np.zeros(ns, np.int64)
                sp[:len(s)] = s
                acc.append(sp)
                dl = np.zeros(ns, np.int64)          # dst local to core slice
                dl[:len(d)] = d[:len(d)] - c * SLICE
                ld.append(dl)
                off = np.full(ns, 255, np.int64)     # 255 => padded slot
                off[:len(d)] = d[:len(d)] - (c * SLICE + b * 128)
                lo.append(off)
        idxA.append(_wrap_idx(np.concatenate(la)))
        idxB.append(_wrap_idx(np.concatenate(lb)))
        dsti.append(_wrap_idx(np.concatenate(ld)))
        doff.append(_slotmajor(np.concatenate(lo), 255, np.float32))

    # --- label edges: 4 groups by (ls-half, ld-half), padded per group ---
    ls0 = edge_label_index[0].astype(np.int64)
    ld0 = edge_label_index[1].astype(np.int64)
    ls = newrow[ls0]
    ld_ = newrow[ld0]
    elpc = (EL + 7) // 8
    groups_sz = np.zeros((8, 4), np.int64)
    per_dec = [[None] * 4 for _ in range(8)]
    for c in range(8):
        lo_, hi_ = c * elpc, min((c + 1) * elpc, EL)
        eidx = np.arange(lo_, hi_)
        g = (ls[eidx] >= HALF).astype(np.int64) * 2 + (ld_[eidx] >= HALF)
        for gi in range(4):
            per_dec[c][gi] = eidx[g == gi]
            groups_sz[c, gi] = len(per_dec[c][gi])
    GSZ = [int(-(-groups_sz[:, gi].max() // DEC_TILE) * DEC_TILE)
           for gi in range(4)]
    SL = sum(GSZ)
    lsw, ldw, tlsw, tldw, slotmap = [], [], [], [], []
    for c in range(8):
        a_ls = np.zeros(SL, np.int64)
        a_ld = np.zeros(SL, np.int64)
        a_tls = np.zeros(SL, np.int64)
        a_tld = np.zeros(SL, np.int64)
        smap = np.full(SL, -1, np.int64)
        pos = 0
        for gi in range(4):
            e = per_dec[c][gi]
            n = len(e)
            a_ls[pos:pos + n] = ls[e] - (HALF if gi >= 2 else 0)
            a_ld[pos:pos + n] = ld_[e] - (HALF if gi % 2 else 0)
            a_tls[pos:pos + n] = types[np.minimum(ls0[e], N - 1)]
            a_tld[pos:pos + n] = types[np.minimum(ld0[e], N - 1)]
            smap[pos:pos + n] = e
            pos += GSZ[gi]
        lsw.append(_wrap_idx(a_ls))
        ldw.append(_wrap_idx(a_ld))
        tlsw.append(_wrap_idx(a_tls))
        tldw.append(_slotmajor(a_tld, 0, np.float32))
        slotmap.append(smap)

    iota128 = np.tile(np.arange(128, dtype=np.float32)[None, :], (128, 1))
    iota384 = np.tile(np.arange(TB_W, dtype=np.float32)[None, :], (128, 1))
    identB = np.eye(128, dtype=bf16)

    in_maps = []
    for c in range(8):
        n0 = c * SLICE
        xT_own = np.zeros((32, SLICE), bf16)
        xT_own[:] = xT_full[:, n0:n0 + SLICE]
        embt_own = _wrap_idx(types_full[n0:n0 + SLICE])
        in_maps.append(dict(
            xT_full=xT_full, embt_full=embt_full,
            xT_own=xT_own, embt_own=embt_own,
            emb_pad=emb_pad,
            RHS1e=RHS[0][0:16], RHS1x=RHS[0][16:48],
            RHS2=RHS[1], RHS3=RHS[2],
            idxA=idxA[c], idxB=idxB[c], dsti=dsti[c], doff=doff[c],
            ls_idx=lsw[c], ld_idx=ldw[c], tls_idx=tlsw[c], tld=tldw[c],
            iota128=iota128, iota384=iota384, identB=identB,
            TBpad=TBpad,
            Wl1a=Wl1[:128].astype(bf16), Wl1b=Wl1[128:].astype(bf16),
            Wl2=Wl2.astype(bf16), bl1=bl1.reshape(64, 1).astype(np.float32),
        ))
    cfg = dict(CA=CA, CB=CB, GSZ=GSZ, SL=SL, newrow=newrow,
               SA=sum(CA) * 128, SB=sum(CB) * 128,
               ST=(sum(CA) + sum(CB)) * 128,
               bl2=float(np.asarray(bl2).reshape(-1)[0]),
               b=[np.asarray(b1), np.asarray(b2), np.asarray(b3)],
               slotmap=slotmap)
    return in_maps, cfg


# ---------------------------------------------------------------- golden ---
def golden(in_maps, cfg):
    """numpy mirror of the device algorithm (layout/precision-accurate)."""
    CA, CB = cfg["CA"], cfg["CB"]

    def f(a):
        return np.asarray(a, np.float32)

    out_all = []
    # T1: full table on every core (identical), bf16
    im0 = in_maps[0]
    RHS1 = np.concatenate([f(im0["RHS1e"]), f(im0["RHS1x"])], axis=0)
    # rowmap[c] = permuted-table rows of core c's local rows 0..SLICE
    nr = cfg["newrow"]
    rowmap = [nr[c * SLICE:(c + 1) * SLICE] for c in range(8)]
    embT = f(im0["emb_pad"])[_unwrap(im0["embt_full"], NP)][:, :EMB]
    x0_full = np.concatenate([embT, f(im0["xT_full"]).T], axis=1)  # [NP,48]
    T = np.zeros((NP, 384), np.float32)
    T[:, :264] = (x0_full @ RHS1).astype(bf16)   # already in permuted order
    # per-core ald tables for L1 (cols 0:8 = als|ald)
    ald_t = []
    for c in range(8):
        im = in_maps[c]
        embo = f(im["emb_pad"])[_unwrap(im["embt_own"], SLICE)][:, :EMB]
        x0 = np.concatenate([embo, f(im["xT_own"]).T], axis=1)
        ald_t.append((x0 @ RHS1[:, 256:264]).astype(bf16).astype(np.float32))

    for li, (Din, HD, H, D, STRIDE, OFF) in enumerate(LCFG):
        PREV = []
        nald_t = []
        Tn = np.zeros((NP, 384), np.float32)
        for c in range(8):
            im = in_maps[c]
            ia = _unwrap(im["idxA"], cfg["SA"])
            ib = _unwrap(im["idxB"], cfg["SB"])
            idt = _unwrap(im["dsti"], cfg["ST"])
            dof = im["doff"].T.reshape(-1)
            xl = np.zeros((SLICE, HD), np.float32)
            pa = pb = pt = 0
            for b in range(NB):
                sA, sB = CA[b] * 128, CB[b] * 128
                gidx = np.concatenate([ia[pa:pa + sA],
                                       ib[pb:pb + sB] + HALF])
                pa += sA; pb += sB
                nsl = sA + sB
                G = T[gidx, :]                     # [nsl, STRIDE]
                GD = ald_t[c][idt[pt:pt + nsl]]
                off = dof[pt:pt + nsl]
                pt += nsl
                S = (off[:, None] == np.arange(128)[None, :]).astype(np.float32)
                als = G[:, OFF:OFF + H]
                ald = GD[:, H:2 * H]
                lg = als + ald
                lg = np.where(lg > 0, lg, 0.2 * lg)
                e = np.exp(lg).astype(bf16).astype(np.float32)
                eex = np.repeat(e, D, axis=1)      # [nsl, HD]
                msg = (G[:, :HD].astype(bf16).astype(np.float32) * eex
                       ).astype(bf16).astype(np.float32)
                num = S.T @ msg                    # [128, HD]
                den = S.T @ e                      # [128, H]
                r = 1.0 / (den + 1e-16)
                xb = (num.reshape(128, H, D) * np.repeat(r, D, 1).reshape(128, H, D)
                      ).reshape(128, HD).astype(bf16).astype(np.float32)
                if li < 2:
                    xb = (np.maximum(xb, 0)
                          + np.exp(np.minimum(xb, 0)).astype(bf16).astype(np.float32)
                          - 1).astype(bf16).astype(np.float32)
                xl[b * 128:(b + 1) * 128] = xb
            PREV.append(xl)
            if li < 2:
                NDin, NHD, NH, ND, NSTRIDE, _ = LCFG[li + 1]
                NW = NHD + 2 * NH
                hrow = (xl.astype(bf16).astype(np.float32)
                        @ f(im[f"RHS{li + 2}"])).astype(bf16).astype(np.float32)
                Tn[rowmap[c], :NW] = hrow
                nald_t.append(hrow[:, NHD:NHD + 2 * NH])
        if li < 2:
            T = Tn
            ald_t = nald_t
    # decode
    TD = np.zeros((NP, TDEC_W), np.float32)
    for c in range(8):
        TD[rowmap[c]] = PREV[c].astype(bf16)
    scores = []
    for c in range(8):
        im = in_maps[c]
        lsv = _unwrap(im["ls_idx"], cfg["SL"])
        ldv = _unwrap(im["ld_idx"], cfg["SL"])
        tlsv = _unwrap(im["tls_idx"], cfg["SL"])
        tldv = im["tld"].T.reshape(-1)
        base_ls = np.zeros(cfg["SL"], np.int64)
        base_ld = np.zeros(cfg["SL"], np.int64)
        pos = 0
        for gi in range(4):
            base_ls[pos:pos + cfg["GSZ"][gi]] = HALF if gi >= 2 else 0
            base_ld[pos:pos + cfg["GSZ"][gi]] = HALF if gi % 2 else 0
            pos += cfg["GSZ"][gi]
        zl = TD[lsv + base_ls]
        zr = TD[ldv + base_ld]
        W1a = f(in_maps[c]["Wl1a"])
        W1b = f(in_maps[c]["Wl1b"])
        h = np.maximum(zl @ W1a + zr @ W1b + in_maps[c]["bl1"].T, 0
                       ).astype(bf16).astype(np.float32)
        base = h @ f(in_maps[c]["Wl2"]) + cfg["bl2"]
        TBg = f(in_maps[c]["TBpad"])[tlsv]
        oh = (tldv[:, None] == np.arange(TB_W)[None, :])
        bias = (TBg * oh).sum(1)
        scores.append(base[:, 0] + bias)
    out = np.zeros((EL, 1), np.float32)
    for c in range(8):
        m = cfg["slotmap"][c] >= 0
        out[cfg["slotmap"][c][m], 0] = scores[c][m]
    return out


def _unwrap(w, n):
    return w[:16, :].T.reshape(-1)[:n].astype(np.int64)


# ----------------------------------------------------------------- device ---
def build(cfg, cut='full', exp='orig', kch=KCH, nq=4, feat='all',
          gbufs=2, prog=PRO_G, gdw=2):  # cuts: noop,prob,pro,l1..l3,full
    import concourse.bacc as bacc
    import concourse.mybir as mybir
    from concourse.tile import TileContext
    dt = mybir.dt
    F = mybir.ActivationFunctionType
    A = mybir.AluOpType
    CA, CB, SL = cfg["CA"], cfg["CB"], cfg["SL"]
    SA, SB, ST = cfg["SA"], cfg["SB"], cfg["ST"]

    nc = bacc.Bacc(num_devices=8, dynamic_dma_scratch_size=57344,
                   num_swdge_queues=nq)
    GMAX = 8  # max 128-chunks per dma_gather call (1024-desc HW cap)

    qctr = [0]

    def nextq():
        qctr[0] += 1
        return qctr[0] % nq

    def gat(out_ap, in_ap, idx_tile, col0, nchunk, elem, q=None, **kw):
        for s0 in range(0, nchunk, GMAX):
            s1 = min(s0 + GMAX, nchunk)
            qn = nextq() if q is None else q
            nc.gpsimd.dma_gather(
                out_ap[:, s0:s1, :], in_ap,
                idx_tile[:, col0 + s0 * 8: col0 + s1 * 8],
                (s1 - s0) * 128, (s1 - s0) * 128, elem, queue_num=qn, **kw)
    inp = {}
    for name, shape, d in [
        ("xT_full", [32, NP], dt.bfloat16),
        ("embt_full", [128, NP // 16], dt.int16),
        ("xT_own", [32, SLICE], dt.bfloat16),
        ("embt_own", [128, SLICE // 16], dt.int16),
        ("emb_pad", [NTYPES, 128], dt.bfloat16),
        ("RHS1e", [16, 264], dt.bfloat16),
        ("RHS1x", [32, 264], dt.bfloat16),
        ("RHS2", [256, 260], dt.bfloat16),
        ("RHS3", [256, 130], dt.bfloat16),
        ("idxA", [128, SA // 16], dt.int16),
        ("idxB", [128, SB // 16], dt.int16),
        ("dsti", [128, ST // 16], dt.int16),
        ("doff", [128, ST // 128], dt.float32),
        ("ls_idx", [128, SL // 16], dt.int16),
        ("ld_idx", [128, SL // 16], dt.int16),
        ("tls_idx", [128, SL // 16], dt.int16),
        ("tld", [128, SL // 128], dt.float32),
        ("iota128", [128, 128], dt.float32),
        ("iota384", [128, TB_W], dt.float32),
        ("identB", [128, 128], dt.bfloat16),
        ("TBpad", [NTYPES, TB_W], dt.bfloat16),
        ("Wl1a", [128, 64], dt.bfloat16),
        ("Wl1b", [128, 64], dt.bfloat16),
        ("Wl2", [64, 1], dt.bfloat16),
        ("bl1", [64, 1], dt.float32),
    ]:
        inp[name] = nc.dram_tensor(name, shape, d, kind="ExternalInput")
    score_out = nc.dram_tensor("score", [SL, 1], dt.float32, kind="ExternalOutput")

    # chunked slice tensors (distinct per collective chunk, no false WAR deps)
    cb0 = (NB // KCH) * 128                      # rows in chunk 0
    crows = [cb0, SLICE - cb0]
    sl_t = {}
    for l, w in ((1, 384), (2, 256)):
        sl_t[l] = [nc.dram_tensor(f"slice{l}_{k}", [crows[k], w], dt.bfloat16,
                                  kind="Internal") for k in range(KCH)]
    sl_d = [nc.dram_tensor(f"sliceD_{k}", [crows[k], TDEC_W], dt.bfloat16,
                           kind="Internal") for k in range(KCH)]
    T1loc = nc.dram_tensor("T1loc", [NP, 384], dt.bfloat16, kind="Internal")
    T_t = {l: nc.dram_tensor(f"T{l}", [NP, w], dt.bfloat16, kind="Internal",
                             addr_space="Shared")
           for l, w in ((1, 384), (2, 256))}
    T_d = nc.dram_tensor("TD", [NP, TDEC_W], dt.bfloat16,
                         kind="Internal", addr_space="Shared")
    sl_ald = [nc.dram_tensor(f"ald{l}", [SLICE, 128], dt.bfloat16,
                             kind="Internal") for l in range(3)]

    with TileContext(nc, num_cores=8) as tc:
        with tc.tile_pool(name="const", bufs=1) as cpool, \
             tc.tile_pool(name="work", bufs=2) as wpool, \
             tc.tile_pool(name="gat", bufs=gbufs) as gpool, \
             tc.tile_pool(name="psum", bufs=2, space="PSUM") as ppool, \
             tc.tile_pool(name="psum1", bufs=1, space="PSUM") as ppool1:
            # ---- resident constants / indices ----
            def load(name, shape, d):
                t = cpool.tile(shape, d, tag=name)
                nc.sync.dma_start(t[:], inp[name][:])
                return t
            idxA = load("idxA", [128, SA // 16], dt.int16)
            idxB = load("idxB", [128, SB // 16], dt.int16)
            dsti = load("dsti", [128, ST // 16], dt.int16)
            doff = load("doff", [128, ST // 128], dt.float32)
            iota = load("iota128", [128, 128], dt.float32)
            identB = load("identB", [128, 128], dt.bfloat16)
            RHS1e = load("RHS1e", [16, 264], dt.bfloat16)
            RHS1x = load("RHS1x", [32, 264], dt.bfloat16)
            RHSs = [None]
            for l, w in ((2, 260), (3, 130)):
                t = cpool.tile([128, 2, w], dt.bfloat16, tag=f"RHS{l}")
                nc.sync.dma_start(
                    t[:], inp[f"RHS{l}"][:].rearrange("(k p) w -> p k w", p=128))
                RHSs.append(t)
            embt_full = load("embt_full", [128, NP // 16], dt.int16)
            embt_own = load("embt_own", [128, SLICE // 16], dt.int16)

            # ---- prologue (b): own-slice ald table for L1 (cols 0:8) ----
            GN = prog * 128
            for g0 in (range(0, NB, prog) if cut != 'noop' else []):
                g1 = min(g0 + prog, NB)
                nblk = g1 - g0
                embT = wpool.tile([128, 1, GN], dt.bfloat16, tag="embT")
                nc.gpsimd.dma_gather(
                    embT[:, :, 0:nblk * 128], inp["emb_pad"][:],
                    embt_own[:, g0 * 8:g1 * 8], nblk * 128, nblk * 128, 128,
                    transpose=True, queue_num=nextq())
                xTg = wpool.tile([32, GN], dt.bfloat16, tag="xTg")
                nc.sync.dma_start(xTg[:, 0:nblk * 128],
                                  inp["xT_own"][:, g0 * 128:g1 * 128])
                for j in range(nblk):
                    pn8 = ppool1.tile([128, 8], dt.float32, tag="pn")
                    nc.tensor.matmul(pn8[:], embT[0:16, 0, j * 128:(j + 1) * 128],
                                     RHS1e[:, 256:264], start=True, stop=False)
                    nc.tensor.matmul(pn8[:], xTg[:, j * 128:(j + 1) * 128],
                                     RHS1x[:, 256:264], start=False, stop=True)
                    b = g0 + j
                    r8 = wpool.tile([128, 8], dt.bfloat16, tag="r8")
                    nc.scalar.copy(r8[:], pn8[:])
                    nc.sync.dma_start(sl_ald[0][b * 128:(b + 1) * 128, 0:8],
                                      r8[:])

            # ---- trans-bias (input-only): compute during prologue ----
            iota384 = load("iota384", [128, TB_W], dt.float32)
            tlsi = load("tls_idx", [128, SL // 16], dt.int16)
            tld = load("tld", [128, SL // 128], dt.float32)
            biassb = cpool.tile([128, SL // 128], dt.float32, tag="biassb")
            if cut == 'full':
                pos0 = 0
                for gi in range(4):
                    for t0 in range(pos0, pos0 + cfg["GSZ"][gi], DEC_TILE):
                        TBg = wpool.tile([128, 4, TB_W], dt.bfloat16, tag="TBg")
                        nc.gpsimd.dma_gather(
                            TBg[:], inp["TBpad"][:],
                            tlsi[:, t0 // 16:(t0 + DEC_TILE) // 16],
                            DEC_TILE, DEC_TILE, TB_W, queue_num=nextq())
                        oh = wpool.tile([128, 4, TB_W], dt.bfloat16, tag="oh")
                        nc.vector.tensor_tensor(
                            oh[:],
                            tld[:, t0 // 128: t0 // 128 + 4].unsqueeze(-1)
                                .broadcast_to([128, 4, TB_W]),
                            iota384[:].unsqueeze(1).broadcast_to([128, 4, TB_W]),
                            A.is_equal)
                        nc.vector.tensor_tensor(oh[:], TBg[:], oh[:], A.mult)
                        nc.vector.tensor_reduce(
                            biassb[:, t0 // 128:t0 // 128 + 4], oh[:],
                            mybir.AxisListType.X, A.add)
                    pos0 += cfg["GSZ"][gi]

            # ---- prologue (a): full T1 on every core ----
            for g0 in (range(0, NBF, prog) if cut not in ('prob', 'noop') else []):
                g1 = min(g0 + prog, NBF)
                nblk = g1 - g0
                embT = wpool.tile([128, 1, GN], dt.bfloat16, tag="embT")
                nc.gpsimd.dma_gather(
                    embT[:, :, 0:nblk * 128], inp["emb_pad"][:],
                    embt_full[:, g0 * 8:g1 * 8], nblk * 128, nblk * 128, 128,
                    transpose=True, queue_num=nextq())
                xTg = wpool.tile([32, GN], dt.bfloat16, tag="xTg")
                nc.sync.dma_start(xTg[:, 0:nblk * 128],
                                  inp["xT_full"][:, g0 * 128:g1 * 128])
                for j in range(nblk):
                    pn = ppool1.tile([128, 264], dt.float32, tag="pn")
                    nc.tensor.matmul(pn[:], embT[0:16, 0, j * 128:(j + 1) * 128],
                                     RHS1e[:], start=True, stop=False)
                    nc.tensor.matmul(pn[:], xTg[:, j * 128:(j + 1) * 128],
                                     RHS1x[:], start=False, stop=True)
                    row = wpool.tile([128, 264], dt.bfloat16, tag="rowT1")
                    nc.scalar.copy(row[:], pn[:])
                    b = g0 + j
                    nc.sync.dma_start(T1loc[b * 128:(b + 1) * 128, 0:264], row[:])

            # ---- three GAT layers ----
            NL = {'noop': 0, 'prob': 0, 'pro': 0, 'l1': 1, 'l2': 2, 'l3': 3, 'full': 3}[cut]
            for li, (Din, HD, H, D, STRIDE, OFF) in enumerate(LCFG[:NL]):
                RW = HD + H
                T_src = T1loc if li == 0 else T_t[li]
                pa = pb = pt_ = 0
                gdt = {}
                for b in range(NB):
                    cA, cB = CA[b], CB[b]
                    C = cA + cB
                    if b % gdw == 0:
                        wblocks = range(b, min(b + gdw, NB))
                        CW = sum(CA[j] + CB[j] for j in wblocks)
                        GDw = gpool.tile([128, CW, 128], dt.bfloat16, tag="GD")
                        gat(GDw, sl_ald[li][:, :], dsti[:], pt_ // 16, CW, 128)
                        off = 0
                        for j in wblocks:
                            gdt[j] = (GDw, off)
                            off += CA[j] + CB[j]
                    GE = ((STRIDE * 2 // 3 if STRIDE == 384 else STRIDE // 2)
                          if feat == 'g512' else STRIDE)
                    OFFg = OFF if feat != 'g512' else GE - H
                    G = gpool.tile([128, C, GE], dt.bfloat16, tag="G")
                    gat(G, T_src[:, 0:GE], idxA[:], pa // 16, cA, GE,
                        elem_step=STRIDE)
                    if cB:
                        gat(G[:, cA:C, :].rearrange("p c e -> p c e"),
                            T_src[HALF:, 0:GE], idxB[:], pb // 16, cB, GE,
                            elem_step=STRIDE)
                    S = wpool.tile([128, C, 128], dt.bfloat16, tag="S")
                    nc.vector.tensor_tensor(
                        S[:],
                        doff[:, pt_ // 128: pt_ // 128 + C].unsqueeze(-1)
                            .broadcast_to([128, C, 128]),
                        iota[:].unsqueeze(1).broadcast_to([128, C, 128]),
                        A.is_equal)
                    GD = gpool.tile([128, C, 128], dt.bfloat16, tag="GD")
                    gat(GD, sl_ald[li][:, :], dsti[:], pt_ // 16, C, 128)
                    lg = wpool.tile([128, C, H], dt.float32, tag="lg")
                    nc.vector.tensor_tensor(
                        lg[:], G[:, :, OFFg:OFFg + H], GD[:, :, H:2 * H],
                        A.add)
                    lg2 = wpool.tile([128, C, H], dt.float32, tag="lg2")
                    nc.vector.tensor_scalar_mul(lg2[:], lg[:], 0.2)
                    nc.vector.tensor_tensor(lg[:], lg[:], lg2[:], A.max)
                    RT = wpool.tile([128, C, RW], dt.bfloat16, tag="RT")
                    nc.scalar.activation(RT[:, :, HD:HD + H], lg[:], F.Exp)
                    if exp == 'bcast':
                        nc.scalar.activation(
                            RT[:, :, 0:HD].rearrange("p c (h d) -> p c h d", h=H),
                            lg[:].unsqueeze(-1).broadcast_to([128, C, H, D]),
                            F.Exp)
                        nc.vector.tensor_tensor(
                            RT[:, :, 0:HD], RT[:, :, 0:HD], G[:, :, 0:HD], A.mult)
                    else:
                        nc.vector.tensor_tensor(
                            RT[:, :, 0:HD].rearrange("p c (h d) -> p c h d", h=H),
                            G[:, :, 0:HD].rearrange("p c (h d) -> p c h d", h=H),
                            RT[:, :, HD:HD + H].unsqueeze(-1)
                                .broadcast_to([128, C, H, D]),
                            A.mult)
                    pe = ppool.tile([128, RW], dt.float32, tag="pe")
                    NMM = 1 if feat == 'nomm' else C
                    for ch in range(NMM):
                        nc.tensor.matmul(pe[:, 0:RW], S[:, ch, :], RT[:, ch, :],
                                         start=(ch == 0), stop=(ch == NMM - 1))
                    pa += cA * 128
                    pb += cB * 128
                    pt_ += C * 128
                    # ---- finalize + node phase ----
                    den = wpool.tile([128, H], dt.float32, tag="den")
                    nc.vector.tensor_scalar_add(den[:], pe[:, HD:HD + H], 1e-16)
                    rec = wpool.tile([128, H], dt.float32, tag="rec")
                    nc.vector.reciprocal(rec[:], den[:])
                    xo = wpool.tile([128, HD], dt.bfloat16, tag="xo")
                    nc.vector.tensor_tensor(
                        xo[:].rearrange("p (h d) -> p h d", h=H),
                        pe[:, 0:HD].rearrange("p (h d) -> p h d", h=H),
                        rec[:].unsqueeze(-1).broadcast_to([128, H, D]),
                        A.mult)
                    kc_ = 0 if b < NB // KCH else 1
                    rb = b - (0 if kc_ == 0 else NB // KCH)
                    if li < 2:
                        m = wpool.tile([128, HD], dt.bfloat16, tag="melu")
                        nc.vector.tensor_scalar_min(m[:], xo[:], 0.0)
                        e1 = wpool.tile([128, HD], dt.bfloat16, tag="e1")
                        nc.scalar.activation(e1[:], m[:], F.Exp)
                        nc.vector.tensor_scalar_max(xo[:], xo[:], 0.0)
                        nc.vector.tensor_tensor(xo[:], xo[:], e1[:], A.add)
                        nc.vector.tensor_scalar_add(xo[:], xo[:], -1.0)
                        NDin, NHD, NH, ND, NSTRIDE, _ = LCFG[li + 1]
                        NW = NHD + 2 * NH
                        xT = wpool.tile([128, 2, 128], dt.bfloat16, tag="xT")
                        for kc in range(2):
                            ptp = ppool1.tile([128, 128], dt.bfloat16, tag="pt")
                            nc.tensor.transpose(
                                ptp[:], xo[:, kc * 128:(kc + 1) * 128], identB[:])
                            nc.scalar.copy(xT[:, kc, :], ptp[:])
                        pn = ppool1.tile([128, 264], dt.float32, tag="pn")
                        for kc in range(2):
                            nc.tensor.matmul(pn[:, 0:NW], xT[:, kc, :],
                                             RHSs[li + 1][:, kc, :],
                                             start=(kc == 0), stop=(kc == 1))
                        row = wpool.tile([128, NW], dt.bfloat16, tag="row")
                        nc.scalar.copy(row[:], pn[:, 0:NW])
                        nc.sync.dma_start(
                            sl_t[li + 1][kc_][rb * 128:(rb + 1) * 128, 0:NW],
                            row[:])
                        nc.sync.dma_start(
                            sl_ald[li + 1][b * 128:(b + 1) * 128, 0:2 * NH],
                            row[:, NHD:NHD + 2 * NH])
                    else:
                        nc.sync.dma_start(
                            sl_d[kc_][rb * 128:(rb + 1) * 128, :], xo[:])
                    # chunked collective as soon as a chunk's blocks are done
                    cpoints = ([NB // KCH - 1, NB - 1] if kch == 2
                               else [NB - 1, NB - 1])
                    if feat == 'nocoll':
                        cpoints = []
                    for k in [i for i, cp in enumerate(cpoints) if cp == b]:
                        o0 = 0 if k == 0 else 8 * cb0
                        o1 = o0 + 8 * crows[k]
                        if li < 2:
                            nc.gpsimd.collective_compute(
                                "AllGather", mybir.AluOpType.bypass,
                                ins=[sl_t[li + 1][k][:]],
                                outs=[T_t[li + 1][o0:o1, :]],
                                replica_groups=[list(range(8))])
                        else:
                            nc.gpsimd.collective_compute(
                                "AllGather", mybir.AluOpType.bypass,
                                ins=[sl_d[k][:]],
                                outs=[T_d[o0:o1, :]],
                                replica_groups=[list(range(8))])

            # ---- decoder ----
            DECODE = cut == 'full'
            lsi = load("ls_idx", [128, SL // 16], dt.int16)
            ldi = load("ld_idx", [128, SL // 16], dt.int16)
            W1a = load("Wl1a", [128, 64], dt.bfloat16)
            W1b = load("Wl1b", [128, 64], dt.bfloat16)
            W2d = load("Wl2", [64, 1], dt.bfloat16)
            bl1 = load("bl1", [64, 1], dt.float32)
            score_sb = cpool.tile([128, SL // 128], dt.float32, tag="score")
            if not DECODE:
                nc.vector.memset(score_sb[:], 0)
            pos = 0
            for gi in range(4 if DECODE else 0):
                gls, gld = (HALF if gi >= 2 else 0), (HALF if gi % 2 else 0)
                for t0 in range(pos, pos + cfg["GSZ"][gi], DEC_TILE):
                    zl = wpool.tile([128, 1, DEC_TILE], dt.bfloat16, tag="zl")
                    nc.gpsimd.dma_gather(
                        zl[:], T_d[gls:, :], lsi[:, t0 // 16:(t0 + DEC_TILE) // 16],
                        DEC_TILE, DEC_TILE, TDEC_W, transpose=True,
                        queue_num=nextq())
                    zr = wpool.tile([128, 1, DEC_TILE], dt.bfloat16, tag="zr")
                    nc.gpsimd.dma_gather(
                        zr[:], T_d[gld:, :], ldi[:, t0 // 16:(t0 + DEC_TILE) // 16],
                        DEC_TILE, DEC_TILE, TDEC_W, transpose=True,
                        queue_num=nextq())
                    ph = ppool.tile([64, DEC_TILE], dt.float32, tag="pe")
                    nc.tensor.matmul(ph[:], W1a[:], zl[:, 0, :], start=True, stop=False)
                    nc.tensor.matmul(ph[:], W1b[:], zr[:, 0, :], start=False, stop=True)
                    hd = wpool.tile([64, DEC_TILE], dt.bfloat16, tag="hd")
                    nc.scalar.activation(hd[:], ph[:], F.Relu, bias=bl1[:])
                    for sub in range(4):
                        pss = ppool1.tile([128, 1], dt.float32, tag="pn")
                        nc.tensor.matmul(pss[:], hd[:, sub * 128:(sub + 1) * 128],
                                         W2d[:], start=True, stop=True)
                        col = t0 // 128 + sub
                        nc.vector.tensor_tensor(
                            score_sb[:, col:col + 1], pss[:],
                            biassb[:, col:col + 1], A.add)
                pos += cfg["GSZ"][gi]
            if cfg["bl2"] != 0.0:
                nc.vector.tensor_scalar_add(score_sb[:], score_sb[:], cfg["bl2"])
            nc.sync.dma_start(
                score_out[:].rearrange("(c p) o -> p (c o)", p=128), score_sb[:])
    nc.finalize()
    return nc


def kernel(**inputs):
    inputs = {k: np.asarray(v) for k, v in inputs.items()}
    in_maps, cfg = prep(**inputs)
    nc = build(cfg)
    from concourse.bass_utils import run_bass_kernel_spmd
    res = run_bass_kernel_spmd(nc, in_maps, core_ids=list(range(8)))
    out = np.zeros((EL, 1), np.float32)
    for c in range(8):
        sc = res.results[c]["score"][:, 0]
        m = cfg["slotmap"][c] >= 0
        out[cfg["slotmap"][c][m], 0] = sc[m]
    return out


# revision 46
# speedup vs baseline: 2.2352x; 1.6948x over previous
"""Trainium2 Bass kernel for nn_DAGLinkPredictor (3-layer GAT + edge decoder).

Sharding: dst-node-sharded GAT across 8 cores. Edges (incl self-loops) are
sorted by dst and grouped into per-core 128-node dst blocks. Per block:
  - dma_gather pulls [h | als] rows (bf16) for edge sources from the node
    table T_l (two gathers: src < 32768 and >= 32768, int16 gather indices),
  - dst-side [als|ald] comes from compact local DRAM tables (256B rows)
    via dma_gather (an S^T@ald PE-matmul alternative measured 2x slower on
    real HW: per-chunk DVE->PE->Act->PE semaphore chains),
  - a one-hot scatter matrix S (VectorE is_equal) scatter-adds messages into
    PSUM via TensorE,
  - softmax is denominator-style: out[d] = sum_e exp(lrelu(als+ald)) * h[src]
    / sum_e exp(...) (exactly segment-softmax; logits are O(1) so no max).
T1 is computed fully on every core from the replicated input (no collective,
prologue inputs pre-permuted to the chunk-major table layout). T2/T3/TD
slices are AllGathered in 2 contiguous chunks overlapped with the block
loop. Gathers round-robin over 4 SWDGE queues (the serialized per-call
completion wait was the dominant real-HW cost). Node-phase matmuls are
bf16.
Decoder: transpose-gathers of z rows + dense matmuls, trans_bias via
gathered rows x one-hot reduce.
"""
import numpy as np
import ml_dtypes

N = 50000
NP = 50176            # padded nodes: 8 * 6272
SLICE = NP // 8       # 6272 nodes per core
NB = SLICE // 128     # 49 blocks per core
NBF = NP // 128       # 392 blocks in the full table
E = 800000
EL = 100000
HALF = 32768          # int16 gather index limit
NTYPES = 311
EMB = 16
COMB = 48

# layer configs: (Din, HD, H, D, src_elem(row stride), dst_off_elems)
LCFG = [
    (48, 256, 4, 64, 384, 256),
    (256, 256, 2, 128, 384, 256),
    (256, 128, 1, 128, 256, 128),
]
TDEC_W = 128          # decode table row elems (bf16, 256B)
TB_W = 384            # padded trans_bias row (bf16)
DEC_TILE = 512
KCH = 2               # collective chunks per layer
PRO_G = 4             # prologue blocks per gather group

bf16 = ml_dtypes.bfloat16


def _wrap_idx(vals):
    """int16 index array for dma_gather: [128, n/16], wrapped over 16
    partitions and replicated across the 8 gpsimd cores."""
    n = len(vals)
    assert n % 16 == 0
    a = np.zeros((128, n // 16), np.int16)
    v = np.asarray(vals, np.int64)
    assert v.min() >= 0 and v.max() < 32768
    w = v.reshape(n // 16, 16).T.astype(np.int16)  # [16, n/16]
    for g in range(8):
        a[16 * g:16 * g + 16, :] = w
    return a


def _slotmajor(vals, fill, dtype):
    """[128, n/128] array with element (p, c) = vals[c*128+p]."""
    n = len(vals)
    assert n % 128 == 0
    return np.asarray(vals, np.float64).reshape(n // 128, 128).T.astype(dtype)


def prep(x, edge_index, edge_label_index, emb, W1, a_src1, a_dst1, b1,
         W2, a_src2, a_dst2, b2, W3, a_src3, a_dst3, b3,
         Wl1, bl1, Wl2, bl2, trans_bias):
    """Host-side (integer/index + weight-layout) preprocessing."""
    types = x[:, 0].astype(np.int64)

    # --- weight folds: RHS_l = [W_l | W_l@a_src per head | W_l@a_dst] ---
    def fold(W, a_s, a_d, H, D):
        cols_s = np.stack([W[:, h * D:(h + 1) * D] @ a_s[h] for h in range(H)], 1)
        cols_d = np.stack([W[:, h * D:(h + 1) * D] @ a_d[h] for h in range(H)], 1)
        return np.concatenate([W, cols_s, cols_d], 1).astype(bf16)
    RHS = [fold(W1, a_src1, a_dst1, 4, 64),
           fold(W2, a_src2, a_dst2, 2, 128),
           fold(W3, a_src3, a_dst3, 1, 128)]

    emb_pad = np.zeros((NTYPES, 128), bf16)
    emb_pad[:, :EMB] = emb.astype(bf16)
    TBpad = np.zeros((NTYPES, TB_W), bf16)
    TBpad[:, :NTYPES] = trans_bias.astype(bf16)

    # chunk-major row permutation: newrow[g] for gathered tables
    # (chunk0 rows of all cores first, then chunk1) so chunked AllGather
    # outputs are contiguous. T1 uses the same space (prologue inputs are
    # permuted host-side).
    CB0 = (NB // KCH) * 128
    C1R = SLICE - CB0
    gg = np.arange(NP, dtype=np.int64)
    cc_ = gg // SLICE
    rr = gg % SLICE
    newrow = np.where(rr < CB0, cc_ * CB0 + rr,
                      8 * CB0 + cc_ * C1R + (rr - CB0))
    orig = np.argsort(newrow)              # orig[nr] = g

    # full-table transposed features (bf16) + full type index (permuted)
    xT_full = np.zeros((32, NP), bf16)
    xT_full[:, :N] = x[:, 1:33].T.astype(bf16)
    xT_full = np.ascontiguousarray(xT_full[:, orig])
    types_full = np.zeros(NP, np.int64)
    types_full[:N] = types
    embt_full = _wrap_idx(types_full[orig])

    # --- edges: add self loops, sort by dst, bucket per core / block ---
    loops = np.arange(N, dtype=np.int64)
    src = np.concatenate([edge_index[0].astype(np.int64), loops])
    dst = np.concatenate([edge_index[1].astype(np.int64), loops])
    order = np.argsort(dst, kind="stable")
    src, dst = src[order], dst[order]
    src = newrow[src]                      # gather in permuted table space

    blk = dst // 128          # global block id (0..391)
    per = [[[None, None] for _ in range(NB)] for _ in range(8)]
    for c in range(8):
        for b in range(NB):
            gb = c * NB + b
            m = blk == gb
            s, d = src[m], dst[m]
            lo = s < HALF
            per[c][b][0] = (s[lo], d[lo])
            per[c][b][1] = (s[~lo] - HALF, d[~lo])
    CA = [max(1, max((len(per[c][b][0][0]) + 127) // 128 for c in range(8)))
          for b in range(NB)]
    CB = [max((len(per[c][b][1][0]) + 127) // 128 for c in range(8))
          for b in range(NB)]
    assert max(CA) <= 32 and max(CB) <= 32

    idxA, idxB, dsti, doff = [], [], [], []
    for c in range(8):
        la, lb, ld, lo = [], [], [], []
        for b in range(NB):
            for half, (cnt, acc) in (((0), (CA[b], la)), ((1), (CB[b], lb))):
                s, d = per[c][b][half]
                ns = cnt * 128
                sp = np.zeros(ns, np.int64)
                sp[:len(s)] = s
                acc.append(sp)
                dl = np.zeros(ns, np.int64)          # dst local to core slice
                dl[:len(d)] = d[:len(d)] - c * SLICE
                ld.append(dl)
                off = np.full(ns, 255, np.int64)     # 255 => padded slot
                off[:len(d)] = d[:len(d)] - (c * SLICE + b * 128)
                lo.append(off)
        idxA.append(_wrap_idx(np.concatenate(la)))
        idxB.append(_wrap_idx(np.concatenate(lb)))
        dsti.append(_wrap_idx(np.concatenate(ld)))
        doff.append(_slotmajor(np.concatenate(lo), 255, np.float32))

    # --- label edges: 4 groups by (ls-half, ld-half), padded per group ---
    ls0 = edge_label_index[0].astype(np.int64)
    ld0 = edge_label_index[1].astype(np.int64)
    ls = newrow[ls0]
    ld_ = newrow[ld0]
    elpc = (EL + 7) // 8
    groups_sz = np.zeros((8, 4), np.int64)
    per_dec = [[None] * 4 for _ in range(8)]
    for c in range(8):
        lo_, hi_ = c * elpc, min((c + 1) * elpc, EL)
        eidx = np.arange(lo_, hi_)
        g = (ls[eidx] >= HALF).astype(np.int64) * 2 + (ld_[eidx] >= HALF)
        for gi in range(4):
            per_dec[c][gi] = eidx[g == gi]
            groups_sz[c, gi] = len(per_dec[c][gi])
    GSZ = [int(-(-groups_sz[:, gi].max() // DEC_TILE) * DEC_TILE)
           for gi in range(4)]
    SL = sum(GSZ)
    lsw, ldw, tlsw, tldw, slotmap = [], [], [], [], []
    for c in range(8):
        a_ls = np.zeros(SL, np.int64)
        a_ld = np.zeros(SL, np.int64)
        a_tls = np.zeros(SL, np.int64)
        a_tld = np.zeros(SL, np.int64)
        smap = np.full(SL, -1, np.int64)
        pos = 0
        for gi in range(4):
            e = per_dec[c][gi]
            n = len(e)
            a_ls[pos:pos + n] = ls[e] - (HALF if gi >= 2 else 0)
            a_ld[pos:pos + n] = ld_[e] - (HALF if gi % 2 else 0)
            a_tls[pos:pos + n] = types[np.minimum(ls0[e], N - 1)]
            a_tld[pos:pos + n] = types[np.minimum(ld0[e], N - 1)]
            smap[pos:pos + n] = e
            pos += GSZ[gi]
        lsw.append(_wrap_idx(a_ls))
        ldw.append(_wrap_idx(a_ld))
        tlsw.append(_wrap_idx(a_tls))
        tldw.append(_slotmajor(a_tld, 0, np.float32))
        slotmap.append(smap)

    iota128 = np.tile(np.arange(128, dtype=np.float32)[None, :], (128, 1))
    iota384 = np.tile(np.arange(TB_W, dtype=np.float32)[None, :], (128, 1))
    identB = np.eye(128, dtype=bf16)

    in_maps = []
    for c in range(8):
        n0 = c * SLICE
        xT_own = np.zeros((32, SLICE), bf16)
        xT_own[:] = xT_full[:, n0:n0 + SLICE]
        embt_own = _wrap_idx(types_full[n0:n0 + SLICE])
        in_maps.append(dict(
            xT_full=xT_full, embt_full=embt_full,
            xT_own=xT_own, embt_own=embt_own,
            emb_pad=emb_pad,
            RHS1e=RHS[0][0:16], RHS1x=RHS[0][16:48],
            RHS2=RHS[1], RHS3=RHS[2],
            idxA=idxA[c], idxB=idxB[c], dsti=dsti[c], doff=doff[c],
            ls_idx=lsw[c], ld_idx=ldw[c], tls_idx=tlsw[c], tld=tldw[c],
            iota128=iota128, iota384=iota384, identB=identB,
            TBpad=TBpad,
            Wl1a=Wl1[:128].astype(bf16), Wl1b=Wl1[128:].astype(bf16),
            Wl2=Wl2.astype(bf16), bl1=bl1.reshape(64, 1).astype(np.float32),
        ))
    cfg = dict(CA=CA, CB=CB, GSZ=GSZ, SL=SL, newrow=newrow,
               SA=sum(CA) * 128, SB=sum(CB) * 128,
               ST=(sum(CA) + sum(CB)) * 128,
               bl2=float(np.asarray(bl2).reshape(-1)[0]),
               b=[np.asarray(b1), np.asarray(b2), np.asarray(b3)],
               slotmap=slotmap)
    return in_maps, cfg


# ---------------------------------------------------------------- golden ---
def golden(in_maps, cfg):
    """numpy mirror of the device algorithm (layout/precision-accurate)."""
    CA, CB = cfg["CA"], cfg["CB"]

    def f(a):
        return np.asarray(a, np.float32)

    out_all = []
    # T1: full table on every core (identical), bf16
    im0 = in_maps[0]
    RHS1 = np.concatenate([f(im0["RHS1e"]), f(im0["RHS1x"])], axis=0)
    # rowmap[c] = permuted-table rows of core c's local rows 0..SLICE
    nr = cfg["newrow"]
    rowmap = [nr[c * SLICE:(c + 1) * SLICE] for c in range(8)]
    embT = f(im0["emb_pad"])[_unwrap(im0["embt_full"], NP)][:, :EMB]
    x0_full = np.concatenate([embT, f(im0["xT_full"]).T], axis=1)  # [NP,48]
    T = np.zeros((NP, 384), np.float32)
    T[:, :264] = (x0_full @ RHS1).astype(bf16)   # already in permuted order
    # per-core ald tables for L1 (cols 0:8 = als|ald)
    ald_t = []
    for c in range(8):
        im = in_maps[c]
        embo = f(im["emb_pad"])[_unwrap(im["embt_own"], SLICE)][:, :EMB]
        x0 = np.concatenate([embo, f(im["xT_own"]).T], axis=1)
        ald_t.append((x0 @ RHS1[:, 256:264]).astype(bf16).astype(np.float32))

    for li, (Din, HD, H, D, STRIDE, OFF) in enumerate(LCFG):
        PREV = []
        nald_t = []
        Tn = np.zeros((NP, 384), np.float32)
        for c in range(8):
            im = in_maps[c]
            ia = _unwrap(im["idxA"], cfg["SA"])
            ib = _unwrap(im["idxB"], cfg["SB"])
            idt = _unwrap(im["dsti"], cfg["ST"])
            dof = im["doff"].T.reshape(-1)
            xl = np.zeros((SLICE, HD), np.float32)
            pa = pb = pt = 0
            for b in range(NB):
                sA, sB = CA[b] * 128, CB[b] * 128
                gidx = np.concatenate([ia[pa:pa + sA],
                                       ib[pb:pb + sB] + HALF])
                pa += sA; pb += sB
                nsl = sA + sB
                G = T[gidx, :]                     # [nsl, STRIDE]
                GD = ald_t[c][idt[pt:pt + nsl]]
                off = dof[pt:pt + nsl]
                pt += nsl
                S = (off[:, None] == np.arange(128)[None, :]).astype(np.float32)
                als = G[:, OFF:OFF + H]
                ald = GD[:, H:2 * H]
                lg = als + ald
                lg = np.where(lg > 0, lg, 0.2 * lg)
                e = np.exp(lg).astype(bf16).astype(np.float32)
                eex = np.repeat(e, D, axis=1)      # [nsl, HD]
                msg = (G[:, :HD].astype(bf16).astype(np.float32) * eex
                       ).astype(bf16).astype(np.float32)
                num = S.T @ msg                    # [128, HD]
                den = S.T @ e                      # [128, H]
                r = 1.0 / (den + 1e-16)
                xb = (num.reshape(128, H, D) * np.repeat(r, D, 1).reshape(128, H, D)
                      ).reshape(128, HD).astype(bf16).astype(np.float32)
                if li < 2:
                    xb = (np.maximum(xb, 0)
                          + np.exp(np.minimum(xb, 0)).astype(bf16).astype(np.float32)
                          - 1).astype(bf16).astype(np.float32)
                xl[b * 128:(b + 1) * 128] = xb
            PREV.append(xl)
            if li < 2:
                NDin, NHD, NH, ND, NSTRIDE, _ = LCFG[li + 1]
                NW = NHD + 2 * NH
                hrow = (xl.astype(bf16).astype(np.float32)
                        @ f(im[f"RHS{li + 2}"])).astype(bf16).astype(np.float32)
                Tn[rowmap[c], :NW] = hrow
                nald_t.append(hrow[:, NHD:NHD + 2 * NH])
        if li < 2:
            T = Tn
            ald_t = nald_t
    # decode
    TD = np.zeros((NP, TDEC_W), np.float32)
    for c in range(8):
        TD[rowmap[c]] = PREV[c].astype(bf16)
    scores = []
    for c in range(8):
        im = in_maps[c]
        lsv = _unwrap(im["ls_idx"], cfg["SL"])
        ldv = _unwrap(im["ld_idx"], cfg["SL"])
        tlsv = _unwrap(im["tls_idx"], cfg["SL"])
        tldv = im["tld"].T.reshape(-1)
        base_ls = np.zeros(cfg["SL"], np.int64)
        base_ld = np.zeros(cfg["SL"], np.int64)
        pos = 0
        for gi in range(4):
            base_ls[pos:pos + cfg["GSZ"][gi]] = HALF if gi >= 2 else 0
            base_ld[pos:pos + cfg["GSZ"][gi]] = HALF if gi % 2 else 0
            pos += cfg["GSZ"][gi]
        zl = TD[lsv + base_ls]
        zr = TD[ldv + base_ld]
        W1a = f(in_maps[c]["Wl1a"])
        W1b = f(in_maps[c]["Wl1b"])
        h = np.maximum(zl @ W1a + zr @ W1b + in_maps[c]["bl1"].T, 0
                       ).astype(bf16).astype(np.float32)
        base = h @ f(in_maps[c]["Wl2"]) + cfg["bl2"]
        TBg = f(in_maps[c]["TBpad"])[tlsv]
        oh = (tldv[:, None] == np.arange(TB_W)[None, :])
        bias = (TBg * oh).sum(1)
        scores.append(base[:, 0] + bias)
    out = np.zeros((EL, 1), np.float32)
    for c in range(8):
        m = cfg["slotmap"][c] >= 0
        out[cfg["slotmap"][c][m], 0] = scores[c][m]
    return out


def _unwrap(w, n):
    return w[:16, :].T.reshape(-1)[:n].astype(np.int64)


# ----------------------------------------------------------------- device ---
def build(cfg, cut='full', exp='orig', kch=KCH, nq=4, feat='all',
          gbufs=2, prog=PRO_G, gdw=2):  # cuts: noop,prob,pro,l1..l3,full
    import concourse.bacc as bacc
    import concourse.mybir as mybir
    from concourse.tile import TileContext
    dt = mybir.dt
    F = mybir.ActivationFunctionType
    A = mybir.AluOpType
    CA, CB, SL = cfg["CA"], cfg["CB"], cfg["SL"]
    SA, SB, ST = cfg["SA"], cfg["SB"], cfg["ST"]

    nc = bacc.Bacc(num_devices=8, dynamic_dma_scratch_size=40960,
                   num_swdge_queues=nq)
    GMAX = 8  # max 128-chunks per dma_gather call (1024-desc HW cap)

    qctr = [0]

    def nextq():
        qctr[0] += 1
        return qctr[0] % nq

    def gat(out_ap, in_ap, idx_tile, col0, nchunk, elem, q=None, **kw):
        for s0 in range(0, nchunk, GMAX):
            s1 = min(s0 + GMAX, nchunk)
            qn = nextq() if q is None else q
            nc.gpsimd.dma_gather(
                out_ap[:, s0:s1, :], in_ap,
                idx_tile[:, col0 + s0 * 8: col0 + s1 * 8],
                (s1 - s0) * 128, (s1 - s0) * 128, elem, queue_num=qn, **kw)
    inp = {}
    for name, shape, d in [
        ("xT_full", [32, NP], dt.bfloat16),
        ("embt_full", [128, NP // 16], dt.int16),
        ("xT_own", [32, SLICE], dt.bfloat16),
        ("embt_own", [128, SLICE // 16], dt.int16),
        ("emb_pad", [NTYPES, 128], dt.bfloat16),
        ("RHS1e", [16, 264], dt.bfloat16),
        ("RHS1x", [32, 264], dt.bfloat16),
        ("RHS2", [256, 260], dt.bfloat16),
        ("RHS3", [256, 130], dt.bfloat16),
        ("idxA", [128, SA // 16], dt.int16),
        ("idxB", [128, SB // 16], dt.int16),
        ("dsti", [128, ST // 16], dt.int16),
        ("doff", [128, ST // 128], dt.float32),
        ("ls_idx", [128, SL // 16], dt.int16),
        ("ld_idx", [128, SL // 16], dt.int16),
        ("tls_idx", [128, SL // 16], dt.int16),
        ("tld", [128, SL // 128], dt.float32),
        ("iota128", [128, 128], dt.float32),
        ("iota384", [128, TB_W], dt.float32),
        ("identB", [128, 128], dt.bfloat16),
        ("TBpad", [NTYPES, TB_W], dt.bfloat16),
        ("Wl1a", [128, 64], dt.bfloat16),
        ("Wl1b", [128, 64], dt.bfloat16),
        ("Wl2", [64, 1], dt.bfloat16),
        ("bl1", [64, 1], dt.float32),
    ]:
        inp[name] = nc.dram_tensor(name, shape, d, kind="ExternalInput")
    score_out = nc.dram_tensor("score", [SL, 1], dt.float32, kind="ExternalOutput")

    # chunked slice tensors (distinct per collective chunk, no false WAR deps)
    cb0 = (NB // KCH) * 128                      # rows in chunk 0
    crows = [cb0, SLICE - cb0]
    sl_t = {}
    for l, w in ((1, 384), (2, 256)):
        sl_t[l] = [nc.dram_tensor(f"slice{l}_{k}", [crows[k], w], dt.bfloat16,
                                  kind="Internal") for k in range(KCH)]
    sl_d = [nc.dram_tensor(f"sliceD_{k}", [crows[k], TDEC_W], dt.bfloat16,
                           kind="Internal") for k in range(KCH)]
    T1loc = nc.dram_tensor("T1loc", [NP, 384], dt.bfloat16, kind="Internal")
    T_t = {l: nc.dram_tensor(f"T{l}", [NP, w], dt.bfloat16, kind="Internal",
                             addr_space="Shared")
           for l, w in ((1, 384), (2, 256))}
    T_d = nc.dram_tensor("TD", [NP, TDEC_W], dt.bfloat16,
                         kind="Internal", addr_space="Shared")
    sl_ald = [nc.dram_tensor(f"ald{l}", [SLICE, 128], dt.bfloat16,
                             kind="Internal") for l in range(3)]

    with TileContext(nc, num_cores=8) as tc:
        with tc.tile_pool(name="const", bufs=1) as cpool, \
             tc.tile_pool(name="work", bufs=2) as wpool, \
             tc.tile_pool(name="gat", bufs=gbufs) as gpool, \
             tc.tile_pool(name="psum", bufs=2, space="PSUM") as ppool, \
             tc.tile_pool(name="psum1", bufs=1, space="PSUM") as ppool1:
            # ---- resident constants / indices ----
            def load(name, shape, d):
                t = cpool.tile(shape, d, tag=name)
                nc.sync.dma_start(t[:], inp[name][:])
                return t
            idxA = load("idxA", [128, SA // 16], dt.int16)
            idxB = load("idxB", [128, SB // 16], dt.int16)
            dsti = load("dsti", [128, ST // 16], dt.int16)
            doff = load("doff", [128, ST // 128], dt.float32)
            iota = load("iota128", [128, 128], dt.float32)
            identB = load("identB", [128, 128], dt.bfloat16)
            RHS1e = load("RHS1e", [16, 264], dt.bfloat16)
            RHS1x = load("RHS1x", [32, 264], dt.bfloat16)
            RHSs = [None]
            for l, w in ((2, 260), (3, 130)):
                t = cpool.tile([128, 2, w], dt.bfloat16, tag=f"RHS{l}")
                nc.sync.dma_start(
                    t[:], inp[f"RHS{l}"][:].rearrange("(k p) w -> p k w", p=128))
                RHSs.append(t)
            embt_full = load("embt_full", [128, NP // 16], dt.int16)
            embt_own = load("embt_own", [128, SLICE // 16], dt.int16)

            # ---- prologue (b): own-slice ald table for L1 (cols 0:8) ----
            GN = prog * 128
            for g0 in (range(0, NB, prog) if cut != 'noop' else []):
                g1 = min(g0 + prog, NB)
                nblk = g1 - g0
                embT = wpool.tile([128, 1, GN], dt.bfloat16, tag="embT")
                nc.gpsimd.dma_gather(
                    embT[:, :, 0:nblk * 128], inp["emb_pad"][:],
                    embt_own[:, g0 * 8:g1 * 8], nblk * 128, nblk * 128, 128,
                    transpose=True, queue_num=nextq())
                xTg = wpool.tile([32, GN], dt.bfloat16, tag="xTg")
                nc.sync.dma_start(xTg[:, 0:nblk * 128],
                                  inp["xT_own"][:, g0 * 128:g1 * 128])
                for j in range(nblk):
                    pn8 = ppool1.tile([128, 8], dt.float32, tag="pn")
                    nc.tensor.matmul(pn8[:], embT[0:16, 0, j * 128:(j + 1) * 128],
                                     RHS1e[:, 256:264], start=True, stop=False)
                    nc.tensor.matmul(pn8[:], xTg[:, j * 128:(j + 1) * 128],
                                     RHS1x[:, 256:264], start=False, stop=True)
                    b = g0 + j
                    r8 = wpool.tile([128, 8], dt.bfloat16, tag="r8")
                    nc.scalar.copy(r8[:], pn8[:])
                    nc.sync.dma_start(sl_ald[0][b * 128:(b + 1) * 128, 0:8],
                                      r8[:])

            # ---- trans-bias (input-only): compute during prologue ----
            iota384 = load("iota384", [128, TB_W], dt.float32)
            tlsi = load("tls_idx", [128, SL // 16], dt.int16)
            tld = load("tld", [128, SL // 128], dt.float32)
            biassb = cpool.tile([128, SL // 128], dt.float32, tag="biassb")
            if cut == 'full':
                pos0 = 0
                for gi in range(4):
                    for t0 in range(pos0, pos0 + cfg["GSZ"][gi],
                                    DEC_TILE // 2):
                        TBg = wpool.tile([128, 2, TB_W], dt.bfloat16, tag="TBg")
                        nc.gpsimd.dma_gather(
                            TBg[:], inp["TBpad"][:],
                            tlsi[:, t0 // 16:(t0 + DEC_TILE // 2) // 16],
                            DEC_TILE // 2, DEC_TILE // 2, TB_W,
                            queue_num=nextq())
                        oh = wpool.tile([128, 2, TB_W], dt.bfloat16, tag="oh")
                        nc.vector.tensor_tensor(
                            oh[:],
                            tld[:, t0 // 128: t0 // 128 + 2].unsqueeze(-1)
                                .broadcast_to([128, 2, TB_W]),
                            iota384[:].unsqueeze(1).broadcast_to([128, 2, TB_W]),
                            A.is_equal)
                        nc.vector.tensor_tensor(oh[:], TBg[:], oh[:], A.mult)
                        nc.vector.tensor_reduce(
                            biassb[:, t0 // 128:t0 // 128 + 2], oh[:],
                            mybir.AxisListType.X, A.add)
                    pos0 += cfg["GSZ"][gi]

            # ---- prologue (a): full T1 on every core ----
            for g0 in (range(0, NBF, prog) if cut not in ('prob', 'noop') else []):
                g1 = min(g0 + prog, NBF)
                nblk = g1 - g0
                embT = wpool.tile([128, 1, GN], dt.bfloat16, tag="embT")
                nc.gpsimd.dma_gather(
                    embT[:, :, 0:nblk * 128], inp["emb_pad"][:],
                    embt_full[:, g0 * 8:g1 * 8], nblk * 128, nblk * 128, 128,
                    transpose=True, queue_num=nextq())
                xTg = wpool.tile([32, GN], dt.bfloat16, tag="xTg")
                nc.sync.dma_start(xTg[:, 0:nblk * 128],
                                  inp["xT_full"][:, g0 * 128:g1 * 128])
                for j in range(nblk):
                    pn = ppool1.tile([128, 264], dt.float32, tag="pn")
                    nc.tensor.matmul(pn[:], embT[0:16, 0, j * 128:(j + 1) * 128],
                                     RHS1e[:], start=True, stop=False)
                    nc.tensor.matmul(pn[:], xTg[:, j * 128:(j + 1) * 128],
                                     RHS1x[:], start=False, stop=True)
                    row = wpool.tile([128, 264], dt.bfloat16, tag="rowT1")
                    nc.scalar.copy(row[:], pn[:])
                    b = g0 + j
                    nc.sync.dma_start(T1loc[b * 128:(b + 1) * 128, 0:264], row[:])

            # ---- three GAT layers ----
            NL = {'noop': 0, 'prob': 0, 'pro': 0, 'l1': 1, 'l2': 2, 'l3': 3, 'full': 3}[cut]
            for li, (Din, HD, H, D, STRIDE, OFF) in enumerate(LCFG[:NL]):
                RW = HD + H
                T_src = T1loc if li == 0 else T_t[li]
                pa = pb = pt_ = 0
                gdt = {}
                for b in range(NB):
                    cA, cB = CA[b], CB[b]
                    C = cA + cB
                    if b % gdw == 0:
                        wblocks = range(b, min(b + gdw, NB))
                        CW = sum(CA[j] + CB[j] for j in wblocks)
                        cAw = sum(CA[j] for j in wblocks)
                        cBw = sum(CB[j] for j in wblocks)
                        GDw = gpool.tile([128, CW, 128], dt.bfloat16, tag="GD")
                        gat(GDw, sl_ald[li][:, :], dsti[:], pt_ // 16, CW, 128)
                        GAw = gpool.tile([128, cAw, STRIDE], dt.bfloat16,
                                         tag="GA")
                        gat(GAw, T_src[:, :], idxA[:], pa // 16, cAw, STRIDE)
                        GBw = gpool.tile([128, max(cBw, 1), STRIDE],
                                         dt.bfloat16, tag="GB")
                        if cBw:
                            gat(GBw, T_src[HALF:, :], idxB[:], pb // 16, cBw,
                                STRIDE)
                        off = aoff = boff = 0
                        for j in wblocks:
                            gdt[j] = (GDw, off, GAw, aoff, GBw, boff)
                            off += CA[j] + CB[j]
                            aoff += CA[j]
                            boff += CB[j]
                    S = wpool.tile([128, C, 128], dt.bfloat16, tag="S")
                    nc.vector.tensor_tensor(
                        S[:],
                        doff[:, pt_ // 128: pt_ // 128 + C].unsqueeze(-1)
                            .broadcast_to([128, C, 128]),
                        iota[:].unsqueeze(1).broadcast_to([128, C, 128]),
                        A.is_equal)
                    GDw, gdo, GAw, ga0, GBw, gb0 = gdt[b]
                    lg = wpool.tile([128, C, H], dt.float32, tag="lg")
                    nc.vector.tensor_tensor(
                        lg[:, 0:cA, :], GAw[:, ga0:ga0 + cA, OFF:OFF + H],
                        GDw[:, gdo:gdo + cA, H:2 * H], A.add)
                    if cB:
                        nc.vector.tensor_tensor(
                            lg[:, cA:C, :],
                            GBw[:, gb0:gb0 + cB, OFF:OFF + H],
                            GDw[:, gdo + cA:gdo + C, H:2 * H], A.add)
                    lg2 = wpool.tile([128, C, H], dt.float32, tag="lg2")
                    nc.vector.tensor_scalar_mul(lg2[:], lg[:], 0.2)
                    nc.vector.tensor_tensor(lg[:], lg[:], lg2[:], A.max)
                    RT = wpool.tile([128, C, RW], dt.bfloat16, tag="RT")
                    nc.scalar.activation(RT[:, :, HD:HD + H], lg[:], F.Exp)
                    nc.vector.tensor_tensor(
                        RT[:, 0:cA, 0:HD].rearrange(
                            "p c (h d) -> p c h d", h=H),
                        GAw[:, ga0:ga0 + cA, 0:HD].rearrange(
                            "p c (h d) -> p c h d", h=H),
                        RT[:, 0:cA, HD:HD + H].unsqueeze(-1)
                            .broadcast_to([128, cA, H, D]),
                        A.mult)
                    if cB:
                        nc.vector.tensor_tensor(
                            RT[:, cA:C, 0:HD].rearrange(
                                "p c (h d) -> p c h d", h=H),
                            GBw[:, gb0:gb0 + cB, 0:HD].rearrange(
                                "p c (h d) -> p c h d", h=H),
                            RT[:, cA:C, HD:HD + H].unsqueeze(-1)
                                .broadcast_to([128, cB, H, D]),
                            A.mult)
                    pe = ppool.tile([128, RW], dt.float32, tag="pe")
                    NMM = 1 if feat == 'nomm' else C
                    for ch in range(NMM):
                        nc.tensor.matmul(pe[:, 0:RW], S[:, ch, :], RT[:, ch, :],
                                         start=(ch == 0), stop=(ch == NMM - 1))
                    pa += cA * 128
                    pb += cB * 128
                    pt_ += C * 128
                    # ---- finalize + node phase ----
                    den = wpool.tile([128, H], dt.float32, tag="den")
                    nc.vector.tensor_scalar_add(den[:], pe[:, HD:HD + H], 1e-16)
                    rec = wpool.tile([128, H], dt.float32, tag="rec")
                    nc.vector.reciprocal(rec[:], den[:])
                    xo = wpool.tile([128, HD], dt.bfloat16, tag="xo")
                    nc.vector.tensor_tensor(
                        xo[:].rearrange("p (h d) -> p h d", h=H),
                        pe[:, 0:HD].rearrange("p (h d) -> p h d", h=H),
                        rec[:].unsqueeze(-1).broadcast_to([128, H, D]),
                        A.mult)
                    kc_ = 0 if b < NB // KCH else 1
                    rb = b - (0 if kc_ == 0 else NB // KCH)
                    if li < 2:
                        m = wpool.tile([128, HD], dt.bfloat16, tag="melu")
                        nc.vector.tensor_scalar_min(m[:], xo[:], 0.0)
                        e1 = wpool.tile([128, HD], dt.bfloat16, tag="e1")
                        nc.scalar.activation(e1[:], m[:], F.Exp)
                        nc.vector.tensor_scalar_max(xo[:], xo[:], 0.0)
                        nc.vector.tensor_tensor(xo[:], xo[:], e1[:], A.add)
                        nc.vector.tensor_scalar_add(xo[:], xo[:], -1.0)
                        NDin, NHD, NH, ND, NSTRIDE, _ = LCFG[li + 1]
                        NW = NHD + 2 * NH
                        xT = wpool.tile([128, 2, 128], dt.bfloat16, tag="xT")
                        for kc in range(2):
                            ptp = ppool1.tile([128, 128], dt.bfloat16, tag="pt")
                            nc.tensor.transpose(
                                ptp[:], xo[:, kc * 128:(kc + 1) * 128], identB[:])
                            nc.scalar.copy(xT[:, kc, :], ptp[:])
                        pn = ppool1.tile([128, 264], dt.float32, tag="pn")
                        for kc in range(2):
                            nc.tensor.matmul(pn[:, 0:NW], xT[:, kc, :],
                                             RHSs[li + 1][:, kc, :],
                                             start=(kc == 0), stop=(kc == 1))
                        row = wpool.tile([128, NW], dt.bfloat16, tag="row")
                        nc.scalar.copy(row[:], pn[:, 0:NW])
                        nc.sync.dma_start(
                            sl_t[li + 1][kc_][rb * 128:(rb + 1) * 128, 0:NW],
                            row[:])
                        nc.sync.dma_start(
                            sl_ald[li + 1][b * 128:(b + 1) * 128, 0:2 * NH],
                            row[:, NHD:NHD + 2 * NH])
                    else:
                        nc.sync.dma_start(
                            sl_d[kc_][rb * 128:(rb + 1) * 128, :], xo[:])
                    # chunked collective as soon as a chunk's blocks are done
                    cpoints = ([NB // KCH - 1, NB - 1] if kch == 2
                               else [NB - 1, NB - 1])
                    if feat == 'nocoll':
                        cpoints = []
                    for k in [i for i, cp in enumerate(cpoints) if cp == b]:
                        o0 = 0 if k == 0 else 8 * cb0
                        o1 = o0 + 8 * crows[k]
                        if li < 2:
                            nc.gpsimd.collective_compute(
                                "AllGather", mybir.AluOpType.bypass,
                                ins=[sl_t[li + 1][k][:]],
                                outs=[T_t[li + 1][o0:o1, :]],
                                replica_groups=[list(range(8))])
                        else:
                            nc.gpsimd.collective_compute(
                                "AllGather", mybir.AluOpType.bypass,
                                ins=[sl_d[k][:]],
                                outs=[T_d[o0:o1, :]],
                                replica_groups=[list(range(8))])

            # ---- decoder ----
            DECODE = cut == 'full'
            lsi = load("ls_idx", [128, SL // 16], dt.int16)
            ldi = load("ld_idx", [128, SL // 16], dt.int16)
            W1a = load("Wl1a", [128, 64], dt.bfloat16)
            W1b = load("Wl1b", [128, 64], dt.bfloat16)
            W2d = load("Wl2", [64, 1], dt.bfloat16)
            bl1 = load("bl1", [64, 1], dt.float32)
            score_sb = cpool.tile([128, SL // 128], dt.float32, tag="score")
            if not DECODE:
                nc.vector.memset(score_sb[:], 0)
            pos = 0
            for gi in range(4 if DECODE else 0):
                gls, gld = (HALF if gi >= 2 else 0), (HALF if gi % 2 else 0)
                for t0 in range(pos, pos + cfg["GSZ"][gi], DEC_TILE):
                    zl = wpool.tile([128, 1, DEC_TILE], dt.bfloat16, tag="zl")
                    nc.gpsimd.dma_gather(
                        zl[:], T_d[gls:, :], lsi[:, t0 // 16:(t0 + DEC_TILE) // 16],
                        DEC_TILE, DEC_TILE, TDEC_W, transpose=True,
                        queue_num=nextq())
                    zr = wpool.tile([128, 1, DEC_TILE], dt.bfloat16, tag="zr")
                    nc.gpsimd.dma_gather(
                        zr[:], T_d[gld:, :], ldi[:, t0 // 16:(t0 + DEC_TILE) // 16],
                        DEC_TILE, DEC_TILE, TDEC_W, transpose=True,
                        queue_num=nextq())
                    ph = ppool.tile([64, DEC_TILE], dt.float32, tag="pe")
                    nc.tensor.matmul(ph[:], W1a[:], zl[:, 0, :], start=True, stop=False)
                    nc.tensor.matmul(ph[:], W1b[:], zr[:, 0, :], start=False, stop=True)
                    hd = wpool.tile([64, DEC_TILE], dt.bfloat16, tag="hd")
                    nc.scalar.activation(hd[:], ph[:], F.Relu, bias=bl1[:])
                    for sub in range(4):
                        pss = ppool1.tile([128, 1], dt.float32, tag="pn")
                        nc.tensor.matmul(pss[:], hd[:, sub * 128:(sub + 1) * 128],
                                         W2d[:], start=True, stop=True)
                        col = t0 // 128 + sub
                        nc.vector.tensor_tensor(
                            score_sb[:, col:col + 1], pss[:],
                            biassb[:, col:col + 1], A.add)
                pos += cfg["GSZ"][gi]
            if cfg["bl2"] != 0.0:
                nc.vector.tensor_scalar_add(score_sb[:], score_sb[:], cfg["bl2"])
            nc.sync.dma_start(
                score_out[:].rearrange("(c p) o -> p (c o)", p=128), score_sb[:])
    nc.finalize()
    return nc


def kernel(**inputs):
    inputs = {k: np.asarray(v) for k, v in inputs.items()}
    in_maps, cfg = prep(**inputs)
    nc = build(cfg)
    from concourse.bass_utils import run_bass_kernel_spmd
    res = run_bass_kernel_spmd(nc, in_maps, core_ids=list(range(8)))
    out = np.zeros((EL, 1), np.float32)
    for c in range(8):
        sc = res.results[c]["score"][:, 0]
        m = cfg["slotmap"][c] >= 0
        out[cfg["slotmap"][c][m], 0] = sc[m]
    return out
